# revision 12
# baseline (speedup 1.0000x reference)
"""Trainium2 Bass kernel for nn_CDIRecModel (CDI extractor + contrastive loss +
domain-masked AdaNorm), data-parallel over batch on 8 NeuronCores.

Key algebraic restructuring vs the reference:
  - Ki/Vi/Ks/Vs are never materialized.  Since
        softmax(Q @ (H W + b)^T) == softmax((H (W Q))^T)   (bias shift cancels)
    the intra/shared attention needs only kq = Q @ W^T per (d, b), then
        scores[b, l] = <H[b, l, :], kq[b, :]>          (fused DVE dot per l)
        hbar[b, :]   = sum_l softmax_l * H[b, l, :]    (fused DVE mul-add per l)
        h_att        = hbar @ Wv + bv                  (PE)
    This turns ~18 GFLOP of projections into ~0.7 GFLOP of fused vector work
    plus small matmuls, leaving the kernel HBM/DVE bound.
  - The per-(d,b) context mean over L is 50 accumulating PE transpose-matmuls
    (lhsT = H_l, rhs = I) into one PSUM tile -> c^T directly in the layout the
    Q projection wants.
  - Per-domain segment stats (sum h, sum h^2, count) and the contrastive-loss
    partial sum are computed as onehot matmuls into persistent PSUM, packed
    into one [6, 257] tile and AllReduced across the 8 cores for exact parity
    with the full-batch reference statistics.
"""

import sys

sys.path.insert(0, "/opt/trn_rl_repo")

import numpy as np

import concourse.bacc as bacc
import concourse.tile as tile
import concourse.mybir as mybir
from concourse.bass_utils import run_bass_kernel_spmd

# Problem shapes (hardcoded per contract)
D, B, L, LS, DH, DU, QK, CD = 5, 2048, 50, 50, 128, 128, 64, 64
EPS = 1e-5
N_CORES = 8
BC = B // N_CORES          # 256 batch rows per core
P = 128                    # partitions
NBLK = BC // P             # 2 blocks per core
SCALE = 0.125              # 1 / sqrt(QK)

F32 = mybir.dt.float32
AX = mybir.AxisListType
ALU = mybir.AluOpType
ACT = mybir.ActivationFunctionType

_CACHED = None


def _build_program():
    nc = bacc.Bacc("TRN2", target_bir_lowering=False, debug=False,
                   num_devices=N_CORES)

    def inp(name, shape):
        return nc.dram_tensor(name, shape, F32, kind="ExternalInput").ap()

    Hi = inp("Hi", [D, BC, L, DH])
    Hs = inp("Hs", [BC, LS, DH])
    uT = inp("uT", [DU, BC])
    h_in = inp("h_in", [BC, DH])
    oh = inp("oh", [BC, D])
    ohT = inp("ohT", [D, BC])
    Wqu = inp("Wqu", [DU, QK])
    Wqc = inp("Wqc", [DH, QK])
    WkiT = inp("WkiT", [QK, DH])
    WksT = inp("WksT", [QK, DH])
    Wvi = inp("Wvi", [DH, QK])
    Wvs = inp("Wvs", [DH, QK])
    Wf = inp("Wf", [2 * QK, CD])
    Wgbg = inp("Wgbg", [CD + 1, DH])   # [Wgb[:, :DH]; bgb[:DH]]
    Wgbb = inp("Wgbb", [CD + 1, DH])   # [Wgb[:, DH:]; bgb[DH:]]
    bq1 = inp("bq1", [QK, 1])
    bvi1 = inp("bvi1", [QK, 1])
    bvs1 = inp("bvs1", [QK, 1])
    bf1 = inp("bf1", [CD, 1])
    I128 = inp("I128", [P, P])

    out = nc.dram_tensor("out", [BC, DH], F32, kind="ExternalOutput").ap()
    lcon = nc.dram_tensor("lcon", [1, 1], F32, kind="ExternalOutput").ap()

    with tile.TileContext(nc) as tc:
        with (
            tc.tile_pool(name="const", bufs=1) as const,
            tc.tile_pool(name="hbig", bufs=2) as hbig,
            tc.tile_pool(name="hsp", bufs=2) as hsp,
            tc.tile_pool(name="work", bufs=2) as work,
            tc.tile_pool(name="keep", bufs=2) as keep,
            tc.tile_pool(name="pwork", bufs=3, space="PSUM") as pwork,
            tc.tile_pool(name="ppers", bufs=1, space="PSUM") as ppers,
            tc.tile_pool(name="dram", bufs=1, space="DRAM") as dram,
        ):
            # ---- constants / weights to SBUF ----
            def cload(ap_in, shape):
                t = const.tile(shape, F32, tag=ap_in.tensor.name)
                nc.sync.dma_start(out=t[:], in_=ap_in)
                return t

            I_t = cload(I128, [P, P])
            Wqu_t = cload(Wqu, [DU, QK])
            Wqc_t = cload(Wqc, [DH, QK])
            WkiT_t = cload(WkiT, [QK, DH])
            WksT_t = cload(WksT, [QK, DH])
            Wvi_t = cload(Wvi, [DH, QK])
            Wvs_t = cload(Wvs, [DH, QK])
            Wf_t = cload(Wf, [2 * QK, CD])
            Wgbg_t = cload(Wgbg, [CD + 1, DH])
            Wgbb_t = cload(Wgbb, [CD + 1, DH])
            bq1_t = cload(bq1, [QK, 1])
            bvi1_t = cload(bvi1, [QK, 1])
            bvs1_t = cload(bvs1, [QK, 1])
            bf1_t = cload(bf1, [CD, 1])
            ones_t = const.tile([P, 1], F32, tag="ones")
            nc.vector.memset(ones_t[:], 1.0)
            eps5_t = const.tile([D, 1], F32, tag="eps5")
            nc.vector.memset(eps5_t[:], EPS)

            # persistent PSUM accumulators (across both blocks)
            # cols: [0:DH] sum h, [DH:2DH] sum h^2, [2DH] count,
            # [2DH+1] (partition 0 only) contrastive-loss partial
            stats_blk = [ppers.tile([D, 2 * DH + 2], F32, tag=f"stats{i}",
                                    name=f"stats{i}")
                         for i in range(NBLK)]

            gammas, betas = [], []

            for blk in range(NBLK):
                b0 = blk * P
                hs_t = hsp.tile([P, LS, DH], F32, tag="hs")
                nc.sync.dma_start(out=hs_t[:], in_=Hs[b0:b0 + P, :, :])
                uT_t = work.tile([DU, P], F32, tag="uT")
                nc.sync.dma_start(out=uT_t[:], in_=uT[:, b0:b0 + P])
                h_t = keep.tile([P, DH], F32, tag="h")
                nc.sync.dma_start(out=h_t[:], in_=h_in[b0:b0 + P, :])
                oh_t = keep.tile([P, D], F32, tag="oh")
                nc.sync.dma_start(out=oh_t[:], in_=oh[b0:b0 + P, :])

                zT_all = keep.tile([QK, D, P], F32, tag="zT_all")
                cat_all = keep.tile([P, D, 2 * QK], F32, tag="cat_all")
                nsq_i = keep.tile([P, D], F32, tag="nsq_i")
                nsq_s = keep.tile([P, D], F32, tag="nsq_s")
                posd = keep.tile([P, D], F32, tag="posd")

                for d in range(D):
                    hb_t = hbig.tile([P, L, DH], F32, tag="hb")
                    nc.sync.dma_start(out=hb_t[:], in_=Hi[d, b0:b0 + P, :, :])

                    # ---- c^T = (1/L) * sum_l H_l^T  (PE transpose-accumulate)
                    cT_ps = pwork.tile([P, P], F32, tag="pp")
                    for l in range(L):
                        nc.tensor.matmul(cT_ps[:], hb_t[:, l, :], I_t[:],
                                         start=(l == 0), stop=(l == L - 1))
                    cT_sb = work.tile([P, P], F32, tag="cT")
                    nc.scalar.activation(cT_sb[:], cT_ps[:], ACT.Copy,
                                         scale=1.0 / L)

                    # ---- Q^T = Wqu^T u^T + Wqc^T c^T + bq
                    QT_ps = pwork.tile([QK, P], F32, tag="pp")
                    nc.tensor.matmul(QT_ps[:], Wqu_t[:], uT_t[:],
                                     start=True, stop=False)
                    nc.tensor.matmul(QT_ps[:], Wqc_t[:], cT_sb[:],
                                     start=False, stop=True)
                    QT_sb = work.tile([QK, P], F32, tag="QT")
                    nc.vector.tensor_scalar_add(QT_sb[:], QT_ps[:], bq1_t[:])

                    # ---- kq = Q @ Wki^T ; kqs = Q @ Wks^T  (b-partition layout)
                    kq_ps = pwork.tile([P, DH], F32, tag="pp")
                    nc.tensor.matmul(kq_ps[:], QT_sb[:], WkiT_t[:],
                                     start=True, stop=True)
                    kq_sb = work.tile([P, DH], F32, tag="kq")
                    nc.vector.tensor_copy(kq_sb[:], kq_ps[:])
                    kqs_ps = pwork.tile([P, DH], F32, tag="pp")
                    nc.tensor.matmul(kqs_ps[:], QT_sb[:], WksT_t[:],
                                     start=True, stop=True)
                    kqs_sb = work.tile([P, DH], F32, tag="kqs")
                    nc.vector.tensor_copy(kqs_sb[:], kqs_ps[:])

                    # ---- one attention stream (intra or shared)
                    def attn_stream(H_t, nl, kq_t, Wv_t, bv_t, cat_dst, sfx):
                        s = work.tile([P, nl], F32, tag="s" + sfx)
                        junk = work.tile([P, DH], F32, tag="junk" + sfx)
                        for l in range(nl):
                            nc.vector.scalar_tensor_tensor(
                                out=junk[:], in0=H_t[:, l, :], scalar=1.0,
                                in1=kq_t[:], op0=ALU.mult, op1=ALU.mult,
                                accum_out=s[:, l:l + 1])
                        m = work.tile([P, 1], F32, tag="m" + sfx)
                        nc.vector.reduce_max(m[:], s[:], axis=AX.X)
                        negm = work.tile([P, 1], F32, tag="negm" + sfx)
                        nc.vector.tensor_scalar_mul(negm[:], m[:], -SCALE)
                        e = work.tile([P, nl], F32, tag="e" + sfx)
                        den = work.tile([P, 1], F32, tag="den" + sfx)
                        nc.scalar.activation(e[:], s[:], ACT.Exp,
                                             bias=negm[:], scale=SCALE,
                                             accum_out=den[:])
                        rden = work.tile([P, 1], F32, tag="rden" + sfx)
                        nc.vector.reciprocal(rden[:], den[:])
                        acc = work.tile([P, DH], F32, tag="acc" + sfx)
                        nc.vector.memset(acc[:], 0.0)
                        for l in range(nl):
                            nc.vector.scalar_tensor_tensor(
                                out=acc[:], in0=H_t[:, l, :],
                                scalar=e[:, l:l + 1], in1=acc[:],
                                op0=ALU.mult, op1=ALU.add)
                        hbs = work.tile([P, DH], F32, tag="hbs" + sfx)
                        nc.vector.tensor_scalar_mul(hbs[:], acc[:], rden[:])
                        hbT_ps = pwork.tile([P, P], F32, tag="pp")
                        nc.tensor.matmul(hbT_ps[:], hbs[:], I_t[:],
                                         start=True, stop=True)
                        hbT_sb = work.tile([P, P], F32, tag="hbT" + sfx)
                        nc.vector.tensor_copy(hbT_sb[:], hbT_ps[:])
                        hT_ps = pwork.tile([QK, P], F32, tag="pp")
                        nc.tensor.matmul(hT_ps[:], Wv_t[:], hbT_sb[:],
                                         start=True, stop=True)
                        nc.vector.tensor_scalar_add(cat_dst, hT_ps[:], bv_t[:])

                    catT = work.tile([2 * QK, P], F32, tag="catT")
                    attn_stream(hb_t, L, kq_sb, Wvi_t, bvi1_t,
                                catT[0:QK, :], "i")
                    attn_stream(hs_t, LS, kqs_sb, Wvs_t, bvs1_t,
                                catT[QK:2 * QK, :], "s")

                    # ---- z^T = Wf^T cat^T + bf
                    zT_ps = pwork.tile([CD, P], F32, tag="pp")
                    nc.tensor.matmul(zT_ps[:], Wf_t[:], catT[:],
                                     start=True, stop=True)
                    nc.vector.tensor_scalar_add(zT_all[:, d, :], zT_ps[:],
                                                bf1_t[:])

                    # ---- cat back to b-partition layout for the dots
                    cat_ps = pwork.tile([P, P], F32, tag="pp")
                    nc.tensor.matmul(cat_ps[:], catT[:], I_t[:],
                                     start=True, stop=True)
                    nc.vector.tensor_copy(cat_all[:, d, :], cat_ps[:])

                    junkd = work.tile([P, QK], F32, tag="junkd")
                    hi_ap = cat_all[:, d, 0:QK]
                    hs_ap = cat_all[:, d, QK:2 * QK]
                    nc.vector.scalar_tensor_tensor(
                        out=junkd[:], in0=hi_ap, scalar=1.0, in1=hi_ap,
                        op0=ALU.mult, op1=ALU.mult,
                        accum_out=nsq_i[:, d:d + 1])
                    nc.vector.scalar_tensor_tensor(
                        out=junkd[:], in0=hs_ap, scalar=1.0, in1=hs_ap,
                        op0=ALU.mult, op1=ALU.mult,
                        accum_out=nsq_s[:, d:d + 1])
                    nc.vector.scalar_tensor_tensor(
                        out=junkd[:], in0=hi_ap, scalar=1.0, in1=hs_ap,
                        op0=ALU.mult, op1=ALU.mult,
                        accum_out=posd[:, d:d + 1])

                # ================= per-block epilogue =================
                # contrastive: rn = 1/max(sqrt(nsq), 1e-12)
                def recip_norm(nsq, sfx):
                    sq = work.tile([P, D], F32, tag="sq" + sfx)
                    nc.scalar.activation(sq[:], nsq[:], ACT.Sqrt)
                    nc.vector.tensor_scalar_max(sq[:], sq[:], 1e-12)
                    rn = keep.tile([P, D], F32, tag="rn" + sfx)
                    nc.vector.reciprocal(rn[:], sq[:])
                    return rn

                rni = recip_norm(nsq_i, "i")
                rns = recip_norm(nsq_s, "s")

                dots = keep.tile([P, D * D], F32, tag="dots")
                junkd2 = work.tile([P, QK], F32, tag="junkd2")
                for d in range(D):
                    for e2 in range(d, D):
                        nc.vector.scalar_tensor_tensor(
                            out=junkd2[:], in0=cat_all[:, d, 0:QK],
                            scalar=1.0, in1=cat_all[:, e2, 0:QK],
                            op0=ALU.mult, op1=ALU.mult,
                            accum_out=dots[:, d * D + e2:d * D + e2 + 1])
                for d in range(D):
                    for e2 in range(d):
                        nc.vector.tensor_copy(
                            dots[:, d * D + e2:d * D + e2 + 1],
                            dots[:, e2 * D + d:e2 * D + d + 1])
                # sim[b, d, e] = dots * rni[d] * rni[e]
                sim = work.tile([P, D * D], F32, tag="sim")
                for d in range(D):
                    row = slice(d * D, (d + 1) * D)
                    nc.vector.tensor_scalar_mul(sim[:, row], dots[:, row],
                                                rni[:, d:d + 1])
                    nc.vector.tensor_mul(sim[:, row], sim[:, row], rni[:])
                Eall = work.tile([P, D * D], F32, tag="Eall")
                nc.scalar.activation(Eall[:], sim[:], ACT.Exp)
                den_con = work.tile([P, D], F32, tag="den_con")
                nc.vector.reduce_sum(
                    den_con[:],
                    Eall[:].rearrange("p (d e) -> p d e", d=D), axis=AX.X)
                posn = work.tile([P, D], F32, tag="posn")
                nc.vector.tensor_mul(posn[:], posd[:], rni[:])
                nc.vector.tensor_mul(posn[:], posn[:], rns[:])
                EP = work.tile([P, D], F32, tag="EP")
                nc.scalar.activation(EP[:], posn[:], ACT.Exp)
                Rc = work.tile([P, D], F32, tag="Rc")
                nc.vector.tensor_scalar_add(Rc[:], den_con[:], 1e-8)
                nc.vector.reciprocal(Rc[:], Rc[:])
                Fr = work.tile([P, D], F32, tag="Fr")
                nc.vector.tensor_mul(Fr[:], EP[:], Rc[:])
                nc.vector.tensor_scalar_add(Fr[:], Fr[:], 1e-8)
                LG = work.tile([P, D], F32, tag="LG")
                nc.scalar.activation(LG[:], Fr[:], ACT.Ln)
                tsum = work.tile([P, 1], F32, tag="tsum")
                nc.vector.reduce_sum(tsum[:], LG[:], axis=AX.X)
                stats_ps = stats_blk[blk]
                nc.tensor.matmul(stats_ps[0:1, 2 * DH + 1:2 * DH + 2],
                                 tsum[:], ones_t[:], start=True, stop=True)

                # injector partial stats (persistent PSUM accumulate)
                h2 = work.tile([P, DH], F32, tag="h2")
                nc.vector.tensor_mul(h2[:], h_t[:], h_t[:])
                nc.tensor.matmul(stats_ps[:, 0:DH], oh_t[:], h_t[:],
                                 start=True, stop=True)
                nc.tensor.matmul(stats_ps[:, DH:2 * DH], oh_t[:], h2[:],
                                 start=True, stop=True)
                nc.tensor.matmul(stats_ps[:, 2 * DH:2 * DH + 1], oh_t[:],
                                 ones_t[:], start=True, stop=True)

                # z_sel = sum_d onehot[:, d] * z_d   (b-partition)
                zsel = work.tile([P, CD], F32, tag="zsel")
                nc.vector.memset(zsel[:], 0.0)
                for d in range(D):
                    zd_ps = pwork.tile([P, CD], F32, tag="pp")
                    nc.tensor.matmul(zd_ps[:], zT_all[:, d, :],
                                     I_t[0:QK, 0:QK], start=True, stop=True)
                    nc.vector.scalar_tensor_tensor(
                        out=zsel[:], in0=zd_ps[:], scalar=oh_t[:, d:d + 1],
                        in1=zsel[:], op0=ALU.mult, op1=ALU.add)
                zselT_aug = work.tile([CD + 1, P], F32, tag="zselT")
                nc.vector.memset(zselT_aug[CD:CD + 1, :], 1.0)
                zT2_ps = pwork.tile([CD, P], F32, tag="pp")
                nc.tensor.matmul(zT2_ps[:], zsel[:], I_t[:],
                                 start=True, stop=True)
                nc.vector.tensor_copy(zselT_aug[0:CD, :], zT2_ps[:])
                gm_ps = pwork.tile([P, DH], F32, tag="pp")
                nc.tensor.matmul(gm_ps[:], zselT_aug[:], Wgbg_t[:],
                                 start=True, stop=True)
                gm_sb = keep.tile([P, DH], F32, tag="gamma")
                nc.vector.tensor_copy(gm_sb[:], gm_ps[:])
                bt_ps = pwork.tile([P, DH], F32, tag="pp")
                nc.tensor.matmul(bt_ps[:], zselT_aug[:], Wgbb_t[:],
                                 start=True, stop=True)
                bt_sb = keep.tile([P, DH], F32, tag="beta")
                nc.vector.tensor_copy(bt_sb[:], bt_ps[:])
                gammas.append(gm_sb)
                betas.append(bt_sb)

            # ================= collective =================
            AR = keep.tile([D, 2 * DH + 2], F32, tag="AR")
            nc.vector.tensor_copy(AR[:], stats_blk[0][:])
            nc.vector.tensor_add(AR[:], AR[:], stats_blk[1][:])
            bi = dram.tile([D, 2 * DH + 2], F32)
            bo = dram.tile([D, 2 * DH + 2], F32)
            nc.gpsimd.dma_start(bi[:], AR[:])
            nc.gpsimd.collective_compute(
                "AllReduce", ALU.add,
                replica_groups=[list(range(N_CORES))],
                ins=[bi.opt()], outs=[bo.opt()])
            ARG = keep.tile([D, 2 * DH + 2], F32, tag="ARG")
            nc.gpsimd.dma_start(ARG[:], bo[:])

            # ---- global stats -> mean_d / rstd_d ----
            cntc = work.tile([D, 1], F32, tag="cntc")
            nc.vector.tensor_scalar_max(cntc[:], ARG[0:D, 2 * DH:2 * DH + 1],
                                        1.0)
            rc = work.tile([D, 1], F32, tag="rc")
            nc.vector.reciprocal(rc[:], cntc[:])
            mean_d = work.tile([D, DH], F32, tag="mean_d")
            nc.vector.tensor_scalar_mul(mean_d[:], ARG[0:D, 0:DH], rc[:])
            msq = work.tile([D, DH], F32, tag="msq")
            nc.vector.tensor_scalar_mul(msq[:], ARG[0:D, DH:2 * DH], rc[:])
            var_d = work.tile([D, DH], F32, tag="var_d")
            nc.vector.tensor_mul(var_d[:], mean_d[:], mean_d[:])
            nc.vector.tensor_sub(var_d[:], msq[:], var_d[:])
            sd = work.tile([D, DH], F32, tag="sd")
            nc.scalar.activation(sd[:], var_d[:], ACT.Sqrt, bias=eps5_t[:])
            rstd_d = work.tile([D, DH], F32, tag="rstd_d")
            nc.vector.reciprocal(rstd_d[:], sd[:])

            lc_sb = work.tile([1, 1], F32, tag="lc")
            nc.vector.tensor_scalar_mul(lc_sb[:], ARG[0:1, 2 * DH + 1:2 * DH + 2],
                                        -1.0 / (B * D))
            nc.sync.dma_start(out=lcon, in_=lc_sb[:])

            # ---- final assembly per block ----
            for blk in range(NBLK):
                b0 = blk * P
                ohT_t = work.tile([D, P], F32, tag="ohT_f")
                nc.sync.dma_start(out=ohT_t[:], in_=ohT[:, b0:b0 + P])
                h_t2 = work.tile([P, DH], F32, tag="h_f")
                nc.sync.dma_start(out=h_t2[:], in_=h_in[b0:b0 + P, :])
                mu_ps = pwork.tile([P, DH], F32, tag="pp")
                nc.tensor.matmul(mu_ps[:], ohT_t[:], mean_d[:],
                                 start=True, stop=True)
                hm = work.tile([P, DH], F32, tag="hm")
                nc.vector.tensor_sub(hm[:], h_t2[:], mu_ps[:])
                rs_ps = pwork.tile([P, DH], F32, tag="pp")
                nc.tensor.matmul(rs_ps[:], ohT_t[:], rstd_d[:],
                                 start=True, stop=True)
                hn = work.tile([P, DH], F32, tag="hn")
                nc.vector.tensor_mul(hn[:], hm[:], rs_ps[:])
                t1 = work.tile([P, DH], F32, tag="t1")
                nc.vector.tensor_mul(t1[:], hn[:], gammas[blk][:])
                nc.vector.tensor_add(t1[:], t1[:], betas[blk][:])
                nc.vector.tensor_add(t1[:], t1[:], h_t2[:])
                nc.sync.dma_start(out=out[b0:b0 + P, :], in_=t1[:])

    nc.compile()
    return nc


def _get_program():
    global _CACHED
    if _CACHED is None:
        _CACHED = _build_program()
    return _CACHED


def _prep_in_maps(u, H_intra, H_share, h, domain_ids):
    u = np.ascontiguousarray(np.asarray(u, dtype=np.float32))
    H_intra = np.asarray(H_intra, dtype=np.float32)
    H_share = np.asarray(H_share, dtype=np.float32)
    h = np.ascontiguousarray(np.asarray(h, dtype=np.float32))
    ids = np.asarray(domain_ids).astype(np.int64)
    onehot = (ids[:, None] == np.arange(D)[None, :]).astype(np.float32)

    in_maps = []
    for c in range(N_CORES):
        bs = slice(c * BC, (c + 1) * BC)
        in_maps.append({
            "Hi": np.ascontiguousarray(H_intra[:, bs]),
            "Hs": np.ascontiguousarray(H_share[bs]),
            "uT": np.ascontiguousarray(u[bs].T),
            "h_in": h[bs],
            "oh": np.ascontiguousarray(onehot[bs]),
            "ohT": np.ascontiguousarray(onehot[bs].T),
        })
    return in_maps


def _shared_weights(Wq, bq, Wki, bki, Wvi, bvi, Wks, bks, Wvs, bvs,
                    Wf, bf, Wgb, bgb):
    f = lambda x: np.ascontiguousarray(np.asarray(x, dtype=np.float32))
    Wq, Wki, Wvi, Wks, Wvs, Wf, Wgb = map(f, (Wq, Wki, Wvi, Wks, Wvs, Wf, Wgb))
    bq, bvi, bvs, bf, bgb = map(f, (bq, bvi, bvs, bf, bgb))
    return {
        "Wqu": Wq[0:DU],
        "Wqc": Wq[DU:DU + DH],
        "WkiT": np.ascontiguousarray(Wki.T),
        "WksT": np.ascontiguousarray(Wks.T),
        "Wvi": Wvi,
        "Wvs": Wvs,
        "Wf": Wf,
        "Wgbg": np.ascontiguousarray(np.vstack([Wgb[:, 0:DH], bgb[None, 0:DH]])),
        "Wgbb": np.ascontiguousarray(np.vstack([Wgb[:, DH:2 * DH],
                                                bgb[None, DH:2 * DH]])),
        "bq1": bq.reshape(QK, 1),
        "bvi1": bvi.reshape(QK, 1),
        "bvs1": bvs.reshape(QK, 1),
        "bf1": bf.reshape(CD, 1),
        "I128": np.eye(P, dtype=np.float32),
    }


def run(trace=False, **inputs):
    nc = _get_program()
    in_maps = _prep_in_maps(inputs["u"], inputs["H_intra"], inputs["H_share"],
                            inputs["h"], inputs["domain_ids"])
    shared = _shared_weights(
        inputs["Wq"], inputs["bq"], inputs["Wki"], inputs["bki"],
        inputs["Wvi"], inputs["bvi"], inputs["Wks"], inputs["bks"],
        inputs["Wvs"], inputs["bvs"], inputs["Wf"], inputs["bf"],
        inputs["Wgb"], inputs["bgb"])
    for m in in_maps:
        m.update(shared)
    res = run_bass_kernel_spmd(nc, in_maps, list(range(N_CORES)),
                               trace=trace)
    out = np.concatenate([res.results[c]["out"] for c in range(N_CORES)],
                         axis=0)
    l_con = np.float32(res.results[0]["lcon"][0, 0])
    return (out, l_con), res


def kernel(**inputs):
    (out, l_con), _ = run(trace=False, **inputs)
    return out, l_con


# revision 16
# speedup vs baseline: 1.3505x; 1.3505x over previous
"""Trainium2 Bass kernel for nn_CDIRecModel (CDI extractor + contrastive loss +
domain-masked AdaNorm), data-parallel over batch on 8 NeuronCores.

Key algebraic restructuring vs the reference:
  - Ki/Vi/Ks/Vs are never materialized.  Since
        softmax(Q @ (H W + b)^T) == softmax((H (W Q))^T)   (bias shift cancels)
    the intra/shared attention needs only kq = Q @ W^T per (d, b), then
        scores[b, l] = <H[b, l, :], kq[b, :]>          (fused DVE dot per l)
        hbar[b, :]   = sum_l softmax_l * H[b, l, :]    (fused DVE mul-add per l)
        h_att        = hbar @ Wv + bv                  (PE)
    This turns ~18 GFLOP of projections into ~0.7 GFLOP of fused vector work
    plus small matmuls, leaving the kernel HBM/DVE bound.
  - The per-(d,b) context mean over L is 50 accumulating PE transpose-matmuls
    (lhsT = H_l, rhs = I) into one PSUM tile -> c^T directly in the layout the
    Q projection wants.
  - Per-domain segment stats (sum h, sum h^2, count) and the contrastive-loss
    partial sum are computed as onehot matmuls into persistent PSUM, packed
    into one [6, 257] tile and AllReduced across the 8 cores for exact parity
    with the full-batch reference statistics.
"""

import sys

sys.path.insert(0, "/opt/trn_rl_repo")

import ml_dtypes
import numpy as np

import concourse.bass as bass
import concourse.bacc as bacc
import concourse.tile as tile
import concourse.mybir as mybir
from concourse.bass_utils import run_bass_kernel_spmd

# Problem shapes (hardcoded per contract)
D, B, L, LS, DH, DU, QK, CD = 5, 2048, 50, 50, 128, 128, 64, 64
EPS = 1e-5
N_CORES = 8
BC = B // N_CORES          # 256 batch rows per core
P = 128                    # partitions
NBLK = BC // P             # 2 blocks per core
SCALE = 0.125              # 1 / sqrt(QK)

F32 = mybir.dt.float32
BF16 = mybir.dt.bfloat16
AX = mybir.AxisListType
ALU = mybir.AluOpType
ACT = mybir.ActivationFunctionType

_CACHED = None


def _build_program():
    nc = bacc.Bacc("TRN2", target_bir_lowering=False, debug=False,
                   num_devices=N_CORES)

    def inp(name, shape):
        return nc.dram_tensor(name, shape, F32, kind="ExternalInput").ap()

    Hi = inp("Hi", [D, BC, L, DH])
    Hs = inp("Hs", [BC, LS, DH])
    uT = inp("uT", [DU, BC])
    h_in = inp("h_in", [BC, DH])
    oh = inp("oh", [BC, D])
    ohT = inp("ohT", [D, BC])
    Wqu = inp("Wqu", [DU, QK])
    Wqc = inp("Wqc", [DH, QK])
    WkiT = inp("WkiT", [QK, DH])
    WksT = inp("WksT", [QK, DH])
    Wvi = inp("Wvi", [DH, QK])
    Wvs = inp("Wvs", [DH, QK])
    Wf = inp("Wf", [2 * QK, CD])
    Wgbg = inp("Wgbg", [CD + 1, DH])   # [Wgb[:, :DH]; bgb[:DH]]
    Wgbb = inp("Wgbb", [CD + 1, DH])   # [Wgb[:, DH:]; bgb[DH:]]
    bq1 = inp("bq1", [QK, 1])
    bvi1 = inp("bvi1", [QK, 1])
    bvs1 = inp("bvs1", [QK, 1])
    bf1 = inp("bf1", [CD, 1])
    I128 = inp("I128", [P, P])
    I128b = nc.dram_tensor("I128b", [P, P], BF16, kind="ExternalInput").ap()

    out = nc.dram_tensor("out", [BC, DH], F32, kind="ExternalOutput").ap()
    lcon = nc.dram_tensor("lcon", [1, 1], F32, kind="ExternalOutput").ap()

    with tile.TileContext(nc) as tc:
        with (
            tc.tile_pool(name="const", bufs=1) as const,
            tc.tile_pool(name="hbig", bufs=2) as hbig,
            tc.tile_pool(name="hsp", bufs=2) as hsp,
            tc.tile_pool(name="work", bufs=2) as work,
            tc.tile_pool(name="keep", bufs=2) as keep,
            tc.tile_pool(name="pwork", bufs=3, space="PSUM") as pwork,
            tc.tile_pool(name="ppers", bufs=1, space="PSUM") as ppers,
            tc.tile_pool(name="dram", bufs=1, space="DRAM") as dram,
        ):
            # ---- constants / weights to SBUF ----
            def cload(ap_in, shape):
                t = const.tile(shape, F32, tag=ap_in.tensor.name)
                nc.sync.dma_start(out=t[:], in_=ap_in)
                return t

            I_t = cload(I128, [P, P])
            Ib_t = const.tile([P, P], BF16, tag="Ib")
            nc.sync.dma_start(out=Ib_t[:], in_=I128b)
            Wqu_t = cload(Wqu, [DU, QK])
            Wqc_t = cload(Wqc, [DH, QK])
            WkiT_t = cload(WkiT, [QK, DH])
            WksT_t = cload(WksT, [QK, DH])
            Wvi_t = cload(Wvi, [DH, QK])
            Wvs_t = cload(Wvs, [DH, QK])
            Wf_t = cload(Wf, [2 * QK, CD])
            Wgbg_t = cload(Wgbg, [CD + 1, DH])
            Wgbb_t = cload(Wgbb, [CD + 1, DH])
            bq1_t = cload(bq1, [QK, 1])
            bvi1_t = cload(bvi1, [QK, 1])
            bvs1_t = cload(bvs1, [QK, 1])
            bf1_t = cload(bf1, [CD, 1])
            ones_t = const.tile([P, 1], F32, tag="ones")
            nc.vector.memset(ones_t[:], 1.0)
            eps5_t = const.tile([D, 1], F32, tag="eps5")
            nc.vector.memset(eps5_t[:], EPS)

            # persistent PSUM accumulators (across both blocks)
            # cols: [0:DH] sum h, [DH:2DH] sum h^2, [2DH] count,
            # [2DH+1] (partition 0 only) contrastive-loss partial
            stats_blk = [ppers.tile([D, 2 * DH + 2], F32, tag=f"stats{i}",
                                    name=f"stats{i}")
                         for i in range(NBLK)]

            gammas, betas = [], []

            for blk in range(NBLK):
                b0 = blk * P
                hs_t = hsp.tile([P, LS, DH], BF16, tag="hs")
                nc.gpsimd.dma_start(out=hs_t[:], in_=Hs[b0:b0 + P, :, :])
                uT_t = work.tile([DU, P], F32, tag="uT")
                nc.sync.dma_start(out=uT_t[:], in_=uT[:, b0:b0 + P])
                h_t = keep.tile([P, DH], F32, tag="h")
                nc.sync.dma_start(out=h_t[:], in_=h_in[b0:b0 + P, :])
                oh_t = keep.tile([P, D], F32, tag="oh")
                nc.sync.dma_start(out=oh_t[:], in_=oh[b0:b0 + P, :])

                zT_all = keep.tile([QK, D, P], F32, tag="zT_all")
                cat_all = keep.tile([P, D, 2 * QK], F32, tag="cat_all")
                nsq_i = keep.tile([P, D], F32, tag="nsq_i")
                nsq_s = keep.tile([P, D], F32, tag="nsq_s")
                posd = keep.tile([P, D], F32, tag="posd")

                for d in range(D):
                    hb_t = hbig.tile([P, L, DH], BF16, tag="hb")
                    nc.gpsimd.dma_start(out=hb_t[:],
                                        in_=Hi[d, b0:b0 + P, :, :])

                    # ---- c^T = (1/L) * sum_l H_l^T  (PE transpose-accumulate)
                    cT_ps = pwork.tile([P, P], F32, tag="pp")
                    for l in range(L):
                        nc.tensor.matmul(cT_ps[:], hb_t[:, l, :], Ib_t[:],
                                         start=(l == 0), stop=(l == L - 1))
                    cT_sb = work.tile([P, P], F32, tag="cT")
                    nc.scalar.activation(cT_sb[:], cT_ps[:], ACT.Copy,
                                         scale=1.0 / L)

                    # ---- Q^T = Wqu^T u^T + Wqc^T c^T + bq
                    QT_ps = pwork.tile([QK, P], F32, tag="pp")
                    nc.tensor.matmul(QT_ps[:], Wqu_t[:], uT_t[:],
                                     start=True, stop=False)
                    nc.tensor.matmul(QT_ps[:], Wqc_t[:], cT_sb[:],
                                     start=False, stop=True)
                    QT_sb = work.tile([QK, P], F32, tag="QT")
                    nc.vector.tensor_scalar_add(QT_sb[:], QT_ps[:], bq1_t[:])

                    # ---- kq = Q @ Wki^T ; kqs = Q @ Wks^T  (b-partition layout)
                    kq_ps = pwork.tile([P, DH], F32, tag="pp")
                    nc.tensor.matmul(kq_ps[:], QT_sb[:], WkiT_t[:],
                                     start=True, stop=True)
                    kq_sb = work.tile([P, DH], BF16, tag="kq")
                    nc.scalar.activation(kq_sb[:], kq_ps[:], ACT.Identity)
                    kqs_ps = pwork.tile([P, DH], F32, tag="pp")
                    nc.tensor.matmul(kqs_ps[:], QT_sb[:], WksT_t[:],
                                     start=True, stop=True)
                    kqs_sb = work.tile([P, DH], BF16, tag="kqs")
                    nc.scalar.activation(kqs_sb[:], kqs_ps[:], ACT.Identity)

                    # ---- one attention stream (intra or shared)
                    # scores:  tmp = H (bf16) * kq   (kq broadcast over l)
                    #          s[:, l] = reduce_dh tmp
                    # softmax: a = exp((s - max) * SCALE) / den   (fused denom)
                    # hbar^T:  tmp = H * a  -> 50 accumulating PE transposes
                    def attn_stream(H_t, nl, kq_bf, Wv_t, bv_t, cat_dst, sfx):
                        tmp = work.tile([P, nl, DH], BF16, tag="tmp" + sfx)
                        kq_b = bass.AP(tensor=kq_bf[:].tensor,
                                       offset=kq_bf[:].offset,
                                       ap=[kq_bf[:].ap[0], [0, nl],
                                           kq_bf[:].ap[1]])
                        nc.vector.tensor_mul(tmp[:], H_t[:], kq_b)
                        s = work.tile([P, nl], F32, tag="s" + sfx)
                        nc.vector.reduce_sum(s[:], tmp[:], axis=AX.X)
                        m = work.tile([P, 1], F32, tag="m" + sfx)
                        nc.vector.reduce_max(m[:], s[:], axis=AX.X)
                        negm = work.tile([P, 1], F32, tag="negm" + sfx)
                        nc.vector.tensor_scalar_mul(negm[:], m[:], -SCALE)
                        e = work.tile([P, nl], F32, tag="e" + sfx)
                        den = work.tile([P, 1], F32, tag="den" + sfx)
                        nc.scalar.activation(e[:], s[:], ACT.Exp,
                                             bias=negm[:], scale=SCALE,
                                             accum_out=den[:])
                        rden = work.tile([P, 1], F32, tag="rden" + sfx)
                        nc.vector.reciprocal(rden[:], den[:])
                        a_bf = work.tile([P, nl], BF16, tag="a" + sfx)
                        nc.vector.tensor_scalar_mul(a_bf[:], e[:], rden[:])
                        a_b = bass.AP(tensor=a_bf[:].tensor,
                                      offset=a_bf[:].offset,
                                      ap=[a_bf[:].ap[0], a_bf[:].ap[1],
                                          [0, DH]])
                        nc.vector.tensor_mul(tmp[:], H_t[:], a_b)
                        hbT_ps = pwork.tile([P, P], F32, tag="pp")
                        for l in range(nl):
                            nc.tensor.matmul(hbT_ps[:], tmp[:, l, :], Ib_t[:],
                                             start=(l == 0), stop=(l == nl - 1))
                        hbT_sb = work.tile([P, P], F32, tag="hbT" + sfx)
                        nc.scalar.activation(hbT_sb[:], hbT_ps[:], ACT.Identity)
                        hT_ps = pwork.tile([QK, P], F32, tag="pp")
                        nc.tensor.matmul(hT_ps[:], Wv_t[:], hbT_sb[:],
                                         start=True, stop=True)
                        nc.scalar.activation(cat_dst, hT_ps[:], ACT.Identity,
                                             bias=bv_t[:])

                    catT = work.tile([2 * QK, P], F32, tag="catT")
                    attn_stream(hb_t, L, kq_sb, Wvi_t, bvi1_t,
                                catT[0:QK, :], "i")
                    attn_stream(hs_t, LS, kqs_sb, Wvs_t, bvs1_t,
                                catT[QK:2 * QK, :], "s")

                    # ---- z^T = Wf^T cat^T + bf
                    zT_ps = pwork.tile([CD, P], F32, tag="pp")
                    nc.tensor.matmul(zT_ps[:], Wf_t[:], catT[:],
                                     start=True, stop=True)
                    nc.vector.tensor_scalar_add(zT_all[:, d, :], zT_ps[:],
                                                bf1_t[:])

                    # ---- cat back to b-partition layout for the dots
                    cat_ps = pwork.tile([P, P], F32, tag="pp")
                    nc.tensor.matmul(cat_ps[:], catT[:], I_t[:],
                                     start=True, stop=True)
                    nc.vector.tensor_copy(cat_all[:, d, :], cat_ps[:])

                    junkd = work.tile([P, QK], F32, tag="junkd")
                    hi_ap = cat_all[:, d, 0:QK]
                    hs_ap = cat_all[:, d, QK:2 * QK]
                    nc.vector.scalar_tensor_tensor(
                        out=junkd[:], in0=hi_ap, scalar=1.0, in1=hi_ap,
                        op0=ALU.mult, op1=ALU.mult,
                        accum_out=nsq_i[:, d:d + 1])
                    nc.vector.scalar_tensor_tensor(
                        out=junkd[:], in0=hs_ap, scalar=1.0, in1=hs_ap,
                        op0=ALU.mult, op1=ALU.mult,
                        accum_out=nsq_s[:, d:d + 1])
                    nc.vector.scalar_tensor_tensor(
                        out=junkd[:], in0=hi_ap, scalar=1.0, in1=hs_ap,
                        op0=ALU.mult, op1=ALU.mult,
                        accum_out=posd[:, d:d + 1])

                # ================= per-block epilogue =================
                # contrastive: rn = 1/max(sqrt(nsq), 1e-12)
                def recip_norm(nsq, sfx):
                    sq = work.tile([P, D], F32, tag="sq" + sfx)
                    nc.scalar.activation(sq[:], nsq[:], ACT.Sqrt)
                    nc.vector.tensor_scalar_max(sq[:], sq[:], 1e-12)
                    rn = keep.tile([P, D], F32, tag="rn" + sfx)
                    nc.vector.reciprocal(rn[:], sq[:])
                    return rn

                rni = recip_norm(nsq_i, "i")
                rns = recip_norm(nsq_s, "s")

                dots = keep.tile([P, D * D], F32, tag="dots")
                junkd2 = work.tile([P, QK], F32, tag="junkd2")
                for d in range(D):
                    for e2 in range(d, D):
                        nc.vector.scalar_tensor_tensor(
                            out=junkd2[:], in0=cat_all[:, d, 0:QK],
                            scalar=1.0, in1=cat_all[:, e2, 0:QK],
                            op0=ALU.mult, op1=ALU.mult,
                            accum_out=dots[:, d * D + e2:d * D + e2 + 1])
                for d in range(D):
                    for e2 in range(d):
                        nc.vector.tensor_copy(
                            dots[:, d * D + e2:d * D + e2 + 1],
                            dots[:, e2 * D + d:e2 * D + d + 1])
                # sim[b, d, e] = dots * rni[d] * rni[e]
                sim = work.tile([P, D * D], F32, tag="sim")
                for d in range(D):
                    row = slice(d * D, (d + 1) * D)
                    nc.vector.tensor_scalar_mul(sim[:, row], dots[:, row],
                                                rni[:, d:d + 1])
                    nc.vector.tensor_mul(sim[:, row], sim[:, row], rni[:])
                Eall = work.tile([P, D * D], F32, tag="Eall")
                nc.scalar.activation(Eall[:], sim[:], ACT.Exp)
                den_con = work.tile([P, D], F32, tag="den_con")
                nc.vector.reduce_sum(
                    den_con[:],
                    Eall[:].rearrange("p (d e) -> p d e", d=D), axis=AX.X)
                posn = work.tile([P, D], F32, tag="posn")
                nc.vector.tensor_mul(posn[:], posd[:], rni[:])
                nc.vector.tensor_mul(posn[:], posn[:], rns[:])
                EP = work.tile([P, D], F32, tag="EP")
                nc.scalar.activation(EP[:], posn[:], ACT.Exp)
                Rc = work.tile([P, D], F32, tag="Rc")
                nc.vector.tensor_scalar_add(Rc[:], den_con[:], 1e-8)
                nc.vector.reciprocal(Rc[:], Rc[:])
                Fr = work.tile([P, D], F32, tag="Fr")
                nc.vector.tensor_mul(Fr[:], EP[:], Rc[:])
                nc.vector.tensor_scalar_add(Fr[:], Fr[:], 1e-8)
                LG = work.tile([P, D], F32, tag="LG")
                nc.scalar.activation(LG[:], Fr[:], ACT.Ln)
                tsum = work.tile([P, 1], F32, tag="tsum")
                nc.vector.reduce_sum(tsum[:], LG[:], axis=AX.X)
                stats_ps = stats_blk[blk]
                nc.tensor.matmul(stats_ps[0:1, 2 * DH + 1:2 * DH + 2],
                                 tsum[:], ones_t[:], start=True, stop=True)

                # injector partial stats (persistent PSUM accumulate)
                h2 = work.tile([P, DH], F32, tag="h2")
                nc.vector.tensor_mul(h2[:], h_t[:], h_t[:])
                nc.tensor.matmul(stats_ps[:, 0:DH], oh_t[:], h_t[:],
                                 start=True, stop=True)
                nc.tensor.matmul(stats_ps[:, DH:2 * DH], oh_t[:], h2[:],
                                 start=True, stop=True)
                nc.tensor.matmul(stats_ps[:, 2 * DH:2 * DH + 1], oh_t[:],
                                 ones_t[:], start=True, stop=True)

                # z_sel = sum_d onehot[:, d] * z_d   (b-partition)
                zsel = work.tile([P, CD], F32, tag="zsel")
                nc.vector.memset(zsel[:], 0.0)
                for d in range(D):
                    zd_ps = pwork.tile([P, CD], F32, tag="pp")
                    nc.tensor.matmul(zd_ps[:], zT_all[:, d, :],
                                     I_t[0:QK, 0:QK], start=True, stop=True)
                    nc.vector.scalar_tensor_tensor(
                        out=zsel[:], in0=zd_ps[:], scalar=oh_t[:, d:d + 1],
                        in1=zsel[:], op0=ALU.mult, op1=ALU.add)
                zselT_aug = work.tile([CD + 1, P], F32, tag="zselT")
                nc.vector.memset(zselT_aug[CD:CD + 1, :], 1.0)
                zT2_ps = pwork.tile([CD, P], F32, tag="pp")
                nc.tensor.matmul(zT2_ps[:], zsel[:], I_t[:],
                                 start=True, stop=True)
                nc.vector.tensor_copy(zselT_aug[0:CD, :], zT2_ps[:])
                gm_ps = pwork.tile([P, DH], F32, tag="pp")
                nc.tensor.matmul(gm_ps[:], zselT_aug[:], Wgbg_t[:],
                                 start=True, stop=True)
                gm_sb = keep.tile([P, DH], F32, tag="gamma")
                nc.vector.tensor_copy(gm_sb[:], gm_ps[:])
                bt_ps = pwork.tile([P, DH], F32, tag="pp")
                nc.tensor.matmul(bt_ps[:], zselT_aug[:], Wgbb_t[:],
                                 start=True, stop=True)
                bt_sb = keep.tile([P, DH], F32, tag="beta")
                nc.vector.tensor_copy(bt_sb[:], bt_ps[:])
                gammas.append(gm_sb)
                betas.append(bt_sb)

            # ================= collective =================
            AR = keep.tile([D, 2 * DH + 2], F32, tag="AR")
            nc.vector.tensor_copy(AR[:], stats_blk[0][:])
            nc.vector.tensor_add(AR[:], AR[:], stats_blk[1][:])
            bi = dram.tile([D, 2 * DH + 2], F32)
            bo = dram.tile([D, 2 * DH + 2], F32)
            nc.gpsimd.dma_start(bi[:], AR[:])
            nc.gpsimd.collective_compute(
                "AllReduce", ALU.add,
                replica_groups=[list(range(N_CORES))],
                ins=[bi.opt()], outs=[bo.opt()])
            ARG = keep.tile([D, 2 * DH + 2], F32, tag="ARG")
            nc.gpsimd.dma_start(ARG[:], bo[:])

            # ---- global stats -> mean_d / rstd_d ----
            cntc = work.tile([D, 1], F32, tag="cntc")
            nc.vector.tensor_scalar_max(cntc[:], ARG[0:D, 2 * DH:2 * DH + 1],
                                        1.0)
            rc = work.tile([D, 1], F32, tag="rc")
            nc.vector.reciprocal(rc[:], cntc[:])
            mean_d = work.tile([D, DH], F32, tag="mean_d")
            nc.vector.tensor_scalar_mul(mean_d[:], ARG[0:D, 0:DH], rc[:])
            msq = work.tile([D, DH], F32, tag="msq")
            nc.vector.tensor_scalar_mul(msq[:], ARG[0:D, DH:2 * DH], rc[:])
            var_d = work.tile([D, DH], F32, tag="var_d")
            nc.vector.tensor_mul(var_d[:], mean_d[:], mean_d[:])
            nc.vector.tensor_sub(var_d[:], msq[:], var_d[:])
            sd = work.tile([D, DH], F32, tag="sd")
            nc.scalar.activation(sd[:], var_d[:], ACT.Sqrt, bias=eps5_t[:])
            rstd_d = work.tile([D, DH], F32, tag="rstd_d")
            nc.vector.reciprocal(rstd_d[:], sd[:])

            lc_sb = work.tile([1, 1], F32, tag="lc")
            nc.vector.tensor_scalar_mul(lc_sb[:], ARG[0:1, 2 * DH + 1:2 * DH + 2],
                                        -1.0 / (B * D))
            nc.sync.dma_start(out=lcon, in_=lc_sb[:])

            # ---- final assembly per block ----
            for blk in range(NBLK):
                b0 = blk * P
                ohT_t = work.tile([D, P], F32, tag="ohT_f")
                nc.sync.dma_start(out=ohT_t[:], in_=ohT[:, b0:b0 + P])
                h_t2 = work.tile([P, DH], F32, tag="h_f")
                nc.sync.dma_start(out=h_t2[:], in_=h_in[b0:b0 + P, :])
                mu_ps = pwork.tile([P, DH], F32, tag="pp")
                nc.tensor.matmul(mu_ps[:], ohT_t[:], mean_d[:],
                                 start=True, stop=True)
                hm = work.tile([P, DH], F32, tag="hm")
                nc.vector.tensor_sub(hm[:], h_t2[:], mu_ps[:])
                rs_ps = pwork.tile([P, DH], F32, tag="pp")
                nc.tensor.matmul(rs_ps[:], ohT_t[:], rstd_d[:],
                                 start=True, stop=True)
                hn = work.tile([P, DH], F32, tag="hn")
                nc.vector.tensor_mul(hn[:], hm[:], rs_ps[:])
                t1 = work.tile([P, DH], F32, tag="t1")
                nc.vector.tensor_mul(t1[:], hn[:], gammas[blk][:])
                nc.vector.tensor_add(t1[:], t1[:], betas[blk][:])
                nc.vector.tensor_add(t1[:], t1[:], h_t2[:])
                nc.sync.dma_start(out=out[b0:b0 + P, :], in_=t1[:])

    nc.compile()
    return nc


def _get_program():
    global _CACHED
    if _CACHED is None:
        _CACHED = _build_program()
    return _CACHED


def _prep_in_maps(u, H_intra, H_share, h, domain_ids):
    u = np.ascontiguousarray(np.asarray(u, dtype=np.float32))
    H_intra = np.asarray(H_intra, dtype=np.float32)
    H_share = np.asarray(H_share, dtype=np.float32)
    h = np.ascontiguousarray(np.asarray(h, dtype=np.float32))
    ids = np.asarray(domain_ids).astype(np.int64)
    onehot = (ids[:, None] == np.arange(D)[None, :]).astype(np.float32)

    in_maps = []
    for c in range(N_CORES):
        bs = slice(c * BC, (c + 1) * BC)
        in_maps.append({
            "Hi": np.ascontiguousarray(H_intra[:, bs]),
            "Hs": np.ascontiguousarray(H_share[bs]),
            "uT": np.ascontiguousarray(u[bs].T),
            "h_in": h[bs],
            "oh": np.ascontiguousarray(onehot[bs]),
            "ohT": np.ascontiguousarray(onehot[bs].T),
        })
    return in_maps


def _shared_weights(Wq, bq, Wki, bki, Wvi, bvi, Wks, bks, Wvs, bvs,
                    Wf, bf, Wgb, bgb):
    f = lambda x: np.ascontiguousarray(np.asarray(x, dtype=np.float32))
    Wq, Wki, Wvi, Wks, Wvs, Wf, Wgb = map(f, (Wq, Wki, Wvi, Wks, Wvs, Wf, Wgb))
    bq, bvi, bvs, bf, bgb = map(f, (bq, bvi, bvs, bf, bgb))
    return {
        "Wqu": Wq[0:DU],
        "Wqc": Wq[DU:DU + DH],
        "WkiT": np.ascontiguousarray(Wki.T),
        "WksT": np.ascontiguousarray(Wks.T),
        "Wvi": Wvi,
        "Wvs": Wvs,
        "Wf": Wf,
        "Wgbg": np.ascontiguousarray(np.vstack([Wgb[:, 0:DH], bgb[None, 0:DH]])),
        "Wgbb": np.ascontiguousarray(np.vstack([Wgb[:, DH:2 * DH],
                                                bgb[None, DH:2 * DH]])),
        "bq1": bq.reshape(QK, 1),
        "bvi1": bvi.reshape(QK, 1),
        "bvs1": bvs.reshape(QK, 1),
        "bf1": bf.reshape(CD, 1),
        "I128": np.eye(P, dtype=np.float32),
        "I128b": np.eye(P, dtype=np.float32).astype(ml_dtypes.bfloat16),
    }


def run(trace=False, **inputs):
    nc = _get_program()
    in_maps = _prep_in_maps(inputs["u"], inputs["H_intra"], inputs["H_share"],
                            inputs["h"], inputs["domain_ids"])
    shared = _shared_weights(
        inputs["Wq"], inputs["bq"], inputs["Wki"], inputs["bki"],
        inputs["Wvi"], inputs["bvi"], inputs["Wks"], inputs["bks"],
        inputs["Wvs"], inputs["bvs"], inputs["Wf"], inputs["bf"],
        inputs["Wgb"], inputs["bgb"])
    for m in in_maps:
        m.update(shared)
    res = run_bass_kernel_spmd(nc, in_maps, list(range(N_CORES)),
                               trace=trace)
    out = np.concatenate([res.results[c]["out"] for c in range(N_CORES)],
                         axis=0)
    l_con = np.float32(res.results[0]["lcon"][0, 0])
    return (out, l_con), res


def kernel(**inputs):
    (out, l_con), _ = run(trace=False, **inputs)
    return out, l_con


# revision 18
# speedup vs baseline: 1.3738x; 1.0172x over previous
"""Trainium2 Bass kernel for nn_CDIRecModel (CDI extractor + contrastive loss +
domain-masked AdaNorm), data-parallel over batch on 8 NeuronCores.

Key algebraic restructuring vs the reference:
  - Ki/Vi/Ks/Vs are never materialized.  Since
        softmax(Q @ (H W + b)^T) == softmax((H (W Q))^T)   (bias shift cancels)
    the intra/shared attention needs only kq = Q @ W^T per (d, b), then
        scores[b, l] = <H[b, l, :], kq[b, :]>          (fused DVE dot per l)
        hbar[b, :]   = sum_l softmax_l * H[b, l, :]    (fused DVE mul-add per l)
        h_att        = hbar @ Wv + bv                  (PE)
    This turns ~18 GFLOP of projections into ~0.7 GFLOP of fused vector work
    plus small matmuls, leaving the kernel HBM/DVE bound.
  - The per-(d,b) context mean over L is 50 accumulating PE transpose-matmuls
    (lhsT = H_l, rhs = I) into one PSUM tile -> c^T directly in the layout the
    Q projection wants.
  - Per-domain segment stats (sum h, sum h^2, count) and the contrastive-loss
    partial sum are computed as onehot matmuls into persistent PSUM, packed
    into one [6, 257] tile and AllReduced across the 8 cores for exact parity
    with the full-batch reference statistics.
"""

import sys

sys.path.insert(0, "/opt/trn_rl_repo")

import ml_dtypes
import numpy as np

import concourse.bass as bass
import concourse.bacc as bacc
import concourse.tile as tile
import concourse.mybir as mybir
from concourse.bass_utils import run_bass_kernel_spmd

# Problem shapes (hardcoded per contract)
D, B, L, LS, DH, DU, QK, CD = 5, 2048, 50, 50, 128, 128, 64, 64
EPS = 1e-5
N_CORES = 8
BC = B // N_CORES          # 256 batch rows per core
P = 128                    # partitions
NBLK = BC // P             # 2 blocks per core
SCALE = 0.125              # 1 / sqrt(QK)

F32 = mybir.dt.float32
BF16 = mybir.dt.bfloat16
AX = mybir.AxisListType
ALU = mybir.AluOpType
ACT = mybir.ActivationFunctionType

_CACHED = None


def _build_program():
    nc = bacc.Bacc("TRN2", target_bir_lowering=False, debug=False,
                   num_devices=N_CORES)

    def inp(name, shape):
        return nc.dram_tensor(name, shape, F32, kind="ExternalInput").ap()

    Hi = inp("Hi", [D, BC, L, DH])
    Hs = inp("Hs", [BC, LS, DH])
    uT = inp("uT", [DU, BC])
    h_in = inp("h_in", [BC, DH])
    oh = inp("oh", [BC, D])
    ohT = inp("ohT", [D, BC])
    Wqu = inp("Wqu", [DU, QK])
    Wqc = inp("Wqc", [DH, QK])
    WkiT = inp("WkiT", [QK, DH])
    WksT = inp("WksT", [QK, DH])
    Wvi = inp("Wvi", [DH, QK])
    Wvs = inp("Wvs", [DH, QK])
    Wf = inp("Wf", [2 * QK, CD])
    Wgbg = inp("Wgbg", [CD + 1, DH])   # [Wgb[:, :DH]; bgb[:DH]]
    Wgbb = inp("Wgbb", [CD + 1, DH])   # [Wgb[:, DH:]; bgb[DH:]]
    bq1 = inp("bq1", [QK, 1])
    bvi1 = inp("bvi1", [QK, 1])
    bvs1 = inp("bvs1", [QK, 1])
    bf1 = inp("bf1", [CD, 1])
    bfrep = inp("bfrep", [P, CD])
    I128 = inp("I128", [P, P])
    I128b = nc.dram_tensor("I128b", [P, P], BF16, kind="ExternalInput").ap()

    out = nc.dram_tensor("out", [BC, DH], F32, kind="ExternalOutput").ap()
    lcon = nc.dram_tensor("lcon", [1, 1], F32, kind="ExternalOutput").ap()

    with tile.TileContext(nc) as tc:
        with (
            tc.tile_pool(name="const", bufs=1) as const,
            tc.tile_pool(name="hbig", bufs=2) as hbig,
            tc.tile_pool(name="hsp", bufs=2) as hsp,
            tc.tile_pool(name="work", bufs=2) as work,
            tc.tile_pool(name="keep", bufs=2) as keep,
            tc.tile_pool(name="pwork", bufs=6, space="PSUM") as pwork,
            tc.tile_pool(name="ppers", bufs=1, space="PSUM") as ppers,
            tc.tile_pool(name="dram", bufs=1, space="DRAM") as dram,
        ):
            # ---- constants / weights to SBUF ----
            def cload(ap_in, shape):
                t = const.tile(shape, F32, tag=ap_in.tensor.name)
                nc.sync.dma_start(out=t[:], in_=ap_in)
                return t

            I_t = cload(I128, [P, P])
            Ib_t = const.tile([P, P], BF16, tag="Ib")
            nc.sync.dma_start(out=Ib_t[:], in_=I128b)
            Wqu_t = cload(Wqu, [DU, QK])
            Wqc_t = cload(Wqc, [DH, QK])
            WkiT_t = cload(WkiT, [QK, DH])
            WksT_t = cload(WksT, [QK, DH])
            Wvi_t = cload(Wvi, [DH, QK])
            Wvs_t = cload(Wvs, [DH, QK])
            Wf_t = cload(Wf, [2 * QK, CD])
            Wgbg_t = cload(Wgbg, [CD + 1, DH])
            Wgbb_t = cload(Wgbb, [CD + 1, DH])
            bq1_t = cload(bq1, [QK, 1])
            bvi1_t = cload(bvi1, [QK, 1])
            bvs1_t = cload(bvs1, [QK, 1])
            bf1_t = cload(bf1, [CD, 1])
            bfrep_t = cload(bfrep, [P, CD])
            ones_t = const.tile([P, 1], F32, tag="ones")
            nc.vector.memset(ones_t[:], 1.0)
            eps5_t = const.tile([D, 1], F32, tag="eps5")
            nc.vector.memset(eps5_t[:], EPS)

            # persistent PSUM accumulators (across both blocks)
            # cols: [0:DH] sum h, [DH:2DH] sum h^2, [2DH] count,
            # [2DH+1] (partition 0 only) contrastive-loss partial
            stats_blk = [ppers.tile([D, 2 * DH + 2], F32, tag=f"stats{i}",
                                    name=f"stats{i}")
                         for i in range(NBLK)]

            gammas, betas = [], []

            for blk in range(NBLK):
                b0 = blk * P
                hs_t = hsp.tile([P, LS, DH], BF16, tag="hs")
                nc.gpsimd.dma_start(out=hs_t[:], in_=Hs[b0:b0 + P, :, :])
                uT_t = work.tile([DU, P], F32, tag="uT")
                nc.sync.dma_start(out=uT_t[:], in_=uT[:, b0:b0 + P])
                h_t = keep.tile([P, DH], F32, tag="h")
                nc.sync.dma_start(out=h_t[:], in_=h_in[b0:b0 + P, :])
                oh_t = keep.tile([P, D], F32, tag="oh")
                nc.sync.dma_start(out=oh_t[:], in_=oh[b0:b0 + P, :])

                zT_all = keep.tile([QK, D, 2, P], F32, tag="zT_all")
                rdi_all = keep.tile([P, D], F32, tag="rdi_all")
                rds_all = keep.tile([P, D], F32, tag="rds_all")
                cat_all = keep.tile([P, D, 2 * QK], F32, tag="cat_all")
                nsq_i = keep.tile([P, D], F32, tag="nsq_i")
                nsq_s = keep.tile([P, D], F32, tag="nsq_s")
                posd = keep.tile([P, D], F32, tag="posd")

                for d in range(D):
                    hb_t = hbig.tile([P, L, DH], BF16, tag="hb")
                    nc.gpsimd.dma_start(out=hb_t[:],
                                        in_=Hi[d, b0:b0 + P, :, :])

                    # ---- c^T = (1/L) * sum_l H_l^T  (PE transpose-accumulate)
                    cT_ps = pwork.tile([P, P], F32, tag="pp")
                    for l in range(L):
                        nc.tensor.matmul(cT_ps[:], hb_t[:, l, :], Ib_t[:],
                                         start=(l == 0), stop=(l == L - 1))
                    cT_sb = work.tile([P, P], F32, tag="cT")
                    nc.scalar.activation(cT_sb[:], cT_ps[:], ACT.Copy,
                                         scale=1.0 / L)

                    # ---- Q^T = Wqu^T u^T + Wqc^T c^T + bq
                    QT_ps = pwork.tile([QK, P], F32, tag="pp")
                    nc.tensor.matmul(QT_ps[:], Wqu_t[:], uT_t[:],
                                     start=True, stop=False)
                    nc.tensor.matmul(QT_ps[:], Wqc_t[:], cT_sb[:],
                                     start=False, stop=True)
                    QT_sb = work.tile([QK, P], F32, tag="QT")
                    nc.vector.tensor_scalar_add(QT_sb[:], QT_ps[:], bq1_t[:])

                    # ---- kq = Q @ Wki^T ; kqs = Q @ Wks^T  (b-partition layout)
                    kq_ps = pwork.tile([P, DH], F32, tag="pp")
                    nc.tensor.matmul(kq_ps[:], QT_sb[:], WkiT_t[:],
                                     start=True, stop=True)
                    kq_sb = work.tile([P, DH], BF16, tag="kq")
                    nc.scalar.activation(kq_sb[:], kq_ps[:], ACT.Identity)
                    kqs_ps = pwork.tile([P, DH], F32, tag="pp")
                    nc.tensor.matmul(kqs_ps[:], QT_sb[:], WksT_t[:],
                                     start=True, stop=True)
                    kqs_sb = work.tile([P, DH], BF16, tag="kqs")
                    nc.scalar.activation(kqs_sb[:], kqs_ps[:], ACT.Identity)

                    # ---- one attention stream (intra or shared)
                    # scores: tmp = H (bf16) * kq -> s = reduce_dh
                    # e_rep = exp((s - max) * SCALE) broadcast over dh (ACT,
                    #   one op, fused 128*denominator via accum_out)
                    # hbar_raw^T = sum_l (H_l * e_l)^T  (PE accumulate).
                    # Normalization by 1/den is deferred: the contrastive
                    # dots are scale-invariant; only z needs it (see zsel).
                    def attn_stream(H_t, nl, kq_bf, Wv_t, cat_dst, rden_dst,
                                    sfx):
                        tmp = work.tile([P, nl, DH], BF16, tag="tmp")
                        kq_b = bass.AP(tensor=kq_bf[:].tensor,
                                       offset=kq_bf[:].offset,
                                       ap=[kq_bf[:].ap[0], [0, nl],
                                           kq_bf[:].ap[1]])
                        nc.vector.tensor_mul(tmp[:], H_t[:], kq_b)
                        s = work.tile([P, nl], F32, tag="s" + sfx)
                        nc.vector.reduce_sum(s[:], tmp[:], axis=AX.X)
                        m = work.tile([P, 1], F32, tag="m" + sfx)
                        nc.vector.reduce_max(m[:], s[:], axis=AX.X)
                        negm = work.tile([P, 1], F32, tag="negm" + sfx)
                        nc.vector.tensor_scalar_mul(negm[:], m[:], -SCALE)
                        e_rep = work.tile([P, nl, DH], BF16, tag="erep")
                        den = work.tile([P, 1], F32, tag="den" + sfx)
                        s_b = bass.AP(tensor=s[:].tensor, offset=s[:].offset,
                                      ap=[s[:].ap[0], s[:].ap[1], [0, DH]])
                        nc.scalar.activation(e_rep[:], s_b, ACT.Exp,
                                             bias=negm[:], scale=SCALE,
                                             accum_out=den[:])
                        rden = work.tile([P, 1], F32, tag="rden" + sfx)
                        nc.vector.reciprocal(rden[:], den[:])
                        nc.vector.tensor_scalar_mul(rden_dst, rden[:],
                                                    float(DH))
                        nc.vector.tensor_mul(tmp[:], H_t[:], e_rep[:])
                        hbT_ps = pwork.tile([P, P], F32, tag="pp")
                        for l in range(nl):
                            nc.tensor.matmul(hbT_ps[:], tmp[:, l, :], Ib_t[:],
                                             start=(l == 0), stop=(l == nl - 1))
                        hbT_sb = work.tile([P, P], F32, tag="hbT" + sfx)
                        nc.scalar.activation(hbT_sb[:], hbT_ps[:], ACT.Identity)
                        hT_ps = pwork.tile([QK, P], F32, tag="pp")
                        nc.tensor.matmul(hT_ps[:], Wv_t[:], hbT_sb[:],
                                         start=True, stop=True)
                        nc.scalar.activation(cat_dst, hT_ps[:], ACT.Identity)

                    catT = work.tile([2 * QK, P], F32, tag="catT")
                    attn_stream(hb_t, L, kq_sb, Wvi_t,
                                catT[0:QK, :], rdi_all[:, d:d + 1], "i")
                    attn_stream(hs_t, LS, kqs_sb, Wvs_t,
                                catT[QK:2 * QK, :], rds_all[:, d:d + 1], "s")

                    # ---- raw z^T halves (normalization deferred to zsel)
                    zTi_ps = pwork.tile([CD, P], F32, tag="pp")
                    nc.tensor.matmul(zTi_ps[:], Wf_t[0:QK, :], catT[0:QK, :],
                                     start=True, stop=True)
                    nc.scalar.activation(zT_all[:, d, 0, :], zTi_ps[:],
                                         ACT.Identity)
                    zTs_ps = pwork.tile([CD, P], F32, tag="pp")
                    nc.tensor.matmul(zTs_ps[:], Wf_t[QK:2 * QK, :],
                                     catT[QK:2 * QK, :],
                                     start=True, stop=True)
                    nc.scalar.activation(zT_all[:, d, 1, :], zTs_ps[:],
                                         ACT.Identity)

                    # ---- cat back to b-partition layout for the dots
                    cat_ps = pwork.tile([P, P], F32, tag="pp")
                    nc.tensor.matmul(cat_ps[:], catT[:], I_t[:],
                                     start=True, stop=True)
                    nc.vector.tensor_copy(cat_all[:, d, :], cat_ps[:])

                    junkd = work.tile([P, QK], F32, tag="junkd")
                    hi_ap = cat_all[:, d, 0:QK]
                    hs_ap = cat_all[:, d, QK:2 * QK]
                    nc.vector.scalar_tensor_tensor(
                        out=junkd[:], in0=hi_ap, scalar=1.0, in1=hi_ap,
                        op0=ALU.mult, op1=ALU.mult,
                        accum_out=nsq_i[:, d:d + 1])
                    nc.vector.scalar_tensor_tensor(
                        out=junkd[:], in0=hs_ap, scalar=1.0, in1=hs_ap,
                        op0=ALU.mult, op1=ALU.mult,
                        accum_out=nsq_s[:, d:d + 1])
                    nc.vector.scalar_tensor_tensor(
                        out=junkd[:], in0=hi_ap, scalar=1.0, in1=hs_ap,
                        op0=ALU.mult, op1=ALU.mult,
                        accum_out=posd[:, d:d + 1])

                # ================= per-block epilogue =================
                # contrastive: rn = 1/max(sqrt(nsq), 1e-12)
                def recip_norm(nsq, sfx):
                    sq = work.tile([P, D], F32, tag="sq" + sfx)
                    nc.scalar.activation(sq[:], nsq[:], ACT.Sqrt)
                    nc.vector.tensor_scalar_max(sq[:], sq[:], 1e-12)
                    rn = keep.tile([P, D], F32, tag="rn" + sfx)
                    nc.vector.reciprocal(rn[:], sq[:])
                    return rn

                rni = recip_norm(nsq_i, "i")
                rns = recip_norm(nsq_s, "s")

                dots = keep.tile([P, D * D], F32, tag="dots")
                junkd2 = work.tile([P, QK], F32, tag="junkd2")
                for d in range(D):
                    for e2 in range(d, D):
                        nc.vector.scalar_tensor_tensor(
                            out=junkd2[:], in0=cat_all[:, d, 0:QK],
                            scalar=1.0, in1=cat_all[:, e2, 0:QK],
                            op0=ALU.mult, op1=ALU.mult,
                            accum_out=dots[:, d * D + e2:d * D + e2 + 1])
                for d in range(D):
                    for e2 in range(d):
                        nc.vector.tensor_copy(
                            dots[:, d * D + e2:d * D + e2 + 1],
                            dots[:, e2 * D + d:e2 * D + d + 1])
                # sim[b, d, e] = dots * rni[d] * rni[e]
                sim = work.tile([P, D * D], F32, tag="sim")
                for d in range(D):
                    row = slice(d * D, (d + 1) * D)
                    nc.vector.tensor_scalar_mul(sim[:, row], dots[:, row],
                                                rni[:, d:d + 1])
                    nc.vector.tensor_mul(sim[:, row], sim[:, row], rni[:])
                Eall = work.tile([P, D * D], F32, tag="Eall")
                nc.scalar.activation(Eall[:], sim[:], ACT.Exp)
                den_con = work.tile([P, D], F32, tag="den_con")
                nc.vector.reduce_sum(
                    den_con[:],
                    Eall[:].rearrange("p (d e) -> p d e", d=D), axis=AX.X)
                posn = work.tile([P, D], F32, tag="posn")
                nc.vector.tensor_mul(posn[:], posd[:], rni[:])
                nc.vector.tensor_mul(posn[:], posn[:], rns[:])
                EP = work.tile([P, D], F32, tag="EP")
                nc.scalar.activation(EP[:], posn[:], ACT.Exp)
                Rc = work.tile([P, D], F32, tag="Rc")
                nc.vector.tensor_scalar_add(Rc[:], den_con[:], 1e-8)
                nc.vector.reciprocal(Rc[:], Rc[:])
                Fr = work.tile([P, D], F32, tag="Fr")
                nc.vector.tensor_mul(Fr[:], EP[:], Rc[:])
                nc.vector.tensor_scalar_add(Fr[:], Fr[:], 1e-8)
                LG = work.tile([P, D], F32, tag="LG")
                nc.scalar.activation(LG[:], Fr[:], ACT.Ln)
                tsum = work.tile([P, 1], F32, tag="tsum")
                nc.vector.reduce_sum(tsum[:], LG[:], axis=AX.X)
                stats_ps = stats_blk[blk]
                nc.tensor.matmul(stats_ps[0:1, 2 * DH + 1:2 * DH + 2],
                                 tsum[:], ones_t[:], start=True, stop=True)

                # injector partial stats (persistent PSUM accumulate)
                h2 = work.tile([P, DH], F32, tag="h2")
                nc.vector.tensor_mul(h2[:], h_t[:], h_t[:])
                nc.tensor.matmul(stats_ps[:, 0:DH], oh_t[:], h_t[:],
                                 start=True, stop=True)
                nc.tensor.matmul(stats_ps[:, DH:2 * DH], oh_t[:], h2[:],
                                 start=True, stop=True)
                nc.tensor.matmul(stats_ps[:, 2 * DH:2 * DH + 1], oh_t[:],
                                 ones_t[:], start=True, stop=True)

                # z_sel = sum_d onehot[:, d] * (rden_i*z_i + rden_s*z_s)
                #         + bf_rep              (b-partition layout)
                zsel = work.tile([P, CD], F32, tag="zsel")
                nc.vector.memset(zsel[:], 0.0)
                for d in range(D):
                    for st, rall in ((0, rdi_all), (1, rds_all)):
                        zd_ps = pwork.tile([P, CD], F32, tag="pp")
                        nc.tensor.matmul(zd_ps[:], zT_all[:, d, st, :],
                                         I_t[0:QK, 0:QK],
                                         start=True, stop=True)
                        wsel = work.tile([P, 1], F32, tag="wsel")
                        nc.vector.tensor_mul(wsel[:], oh_t[:, d:d + 1],
                                             rall[:, d:d + 1])
                        nc.vector.scalar_tensor_tensor(
                            out=zsel[:], in0=zd_ps[:], scalar=wsel[:],
                            in1=zsel[:], op0=ALU.mult, op1=ALU.add)
                nc.vector.tensor_add(zsel[:], zsel[:], bfrep_t[:])
                zselT_aug = work.tile([CD + 1, P], F32, tag="zselT")
                nc.vector.memset(zselT_aug[CD:CD + 1, :], 1.0)
                zT2_ps = pwork.tile([CD, P], F32, tag="pp")
                nc.tensor.matmul(zT2_ps[:], zsel[:], I_t[:],
                                 start=True, stop=True)
                nc.vector.tensor_copy(zselT_aug[0:CD, :], zT2_ps[:])
                gm_ps = pwork.tile([P, DH], F32, tag="pp")
                nc.tensor.matmul(gm_ps[:], zselT_aug[:], Wgbg_t[:],
                                 start=True, stop=True)
                gm_sb = keep.tile([P, DH], F32, tag="gamma")
                nc.vector.tensor_copy(gm_sb[:], gm_ps[:])
                bt_ps = pwork.tile([P, DH], F32, tag="pp")
                nc.tensor.matmul(bt_ps[:], zselT_aug[:], Wgbb_t[:],
                                 start=True, stop=True)
                bt_sb = keep.tile([P, DH], F32, tag="beta")
                nc.vector.tensor_copy(bt_sb[:], bt_ps[:])
                gammas.append(gm_sb)
                betas.append(bt_sb)

            # ================= collective =================
            AR = keep.tile([D, 2 * DH + 2], F32, tag="AR")
            nc.vector.tensor_copy(AR[:], stats_blk[0][:])
            nc.vector.tensor_add(AR[:], AR[:], stats_blk[1][:])
            bi = dram.tile([D, 2 * DH + 2], F32)
            bo = dram.tile([D, 2 * DH + 2], F32)
            nc.gpsimd.dma_start(bi[:], AR[:])
            nc.gpsimd.collective_compute(
                "AllReduce", ALU.add,
                replica_groups=[list(range(N_CORES))],
                ins=[bi.opt()], outs=[bo.opt()])
            ARG = keep.tile([D, 2 * DH + 2], F32, tag="ARG")
            nc.gpsimd.dma_start(ARG[:], bo[:])

            # ---- global stats -> mean_d / rstd_d ----
            cntc = work.tile([D, 1], F32, tag="cntc")
            nc.vector.tensor_scalar_max(cntc[:], ARG[0:D, 2 * DH:2 * DH + 1],
                                        1.0)
            rc = work.tile([D, 1], F32, tag="rc")
            nc.vector.reciprocal(rc[:], cntc[:])
            mean_d = work.tile([D, DH], F32, tag="mean_d")
            nc.vector.tensor_scalar_mul(mean_d[:], ARG[0:D, 0:DH], rc[:])
            msq = work.tile([D, DH], F32, tag="msq")
            nc.vector.tensor_scalar_mul(msq[:], ARG[0:D, DH:2 * DH], rc[:])
            var_d = work.tile([D, DH], F32, tag="var_d")
            nc.vector.tensor_mul(var_d[:], mean_d[:], mean_d[:])
            nc.vector.tensor_sub(var_d[:], msq[:], var_d[:])
            sd = work.tile([D, DH], F32, tag="sd")
            nc.scalar.activation(sd[:], var_d[:], ACT.Sqrt, bias=eps5_t[:])
            rstd_d = work.tile([D, DH], F32, tag="rstd_d")
            nc.vector.reciprocal(rstd_d[:], sd[:])

            lc_sb = work.tile([1, 1], F32, tag="lc")
            nc.vector.tensor_scalar_mul(lc_sb[:], ARG[0:1, 2 * DH + 1:2 * DH + 2],
                                        -1.0 / (B * D))
            nc.sync.dma_start(out=lcon, in_=lc_sb[:])

            # ---- final assembly per block ----
            for blk in range(NBLK):
                b0 = blk * P
                ohT_t = work.tile([D, P], F32, tag="ohT_f")
                nc.sync.dma_start(out=ohT_t[:], in_=ohT[:, b0:b0 + P])
                h_t2 = work.tile([P, DH], F32, tag="h_f")
                nc.sync.dma_start(out=h_t2[:], in_=h_in[b0:b0 + P, :])
                mu_ps = pwork.tile([P, DH], F32, tag="pp")
                nc.tensor.matmul(mu_ps[:], ohT_t[:], mean_d[:],
                                 start=True, stop=True)
                hm = work.tile([P, DH], F32, tag="hm")
                nc.vector.tensor_sub(hm[:], h_t2[:], mu_ps[:])
                rs_ps = pwork.tile([P, DH], F32, tag="pp")
                nc.tensor.matmul(rs_ps[:], ohT_t[:], rstd_d[:],
                                 start=True, stop=True)
                hn = work.tile([P, DH], F32, tag="hn")
                nc.vector.tensor_mul(hn[:], hm[:], rs_ps[:])
                t1 = work.tile([P, DH], F32, tag="t1")
                nc.vector.tensor_mul(t1[:], hn[:], gammas[blk][:])
                nc.vector.tensor_add(t1[:], t1[:], betas[blk][:])
                nc.vector.tensor_add(t1[:], t1[:], h_t2[:])
                nc.sync.dma_start(out=out[b0:b0 + P, :], in_=t1[:])

    nc.compile()
    return nc


def _get_program():
    global _CACHED
    if _CACHED is None:
        _CACHED = _build_program()
    return _CACHED


def _prep_in_maps(u, H_intra, H_share, h, domain_ids):
    u = np.ascontiguousarray(np.asarray(u, dtype=np.float32))
    H_intra = np.asarray(H_intra, dtype=np.float32)
    H_share = np.asarray(H_share, dtype=np.float32)
    h = np.ascontiguousarray(np.asarray(h, dtype=np.float32))
    ids = np.asarray(domain_ids).astype(np.int64)
    onehot = (ids[:, None] == np.arange(D)[None, :]).astype(np.float32)

    in_maps = []
    for c in range(N_CORES):
        bs = slice(c * BC, (c + 1) * BC)
        in_maps.append({
            "Hi": np.ascontiguousarray(H_intra[:, bs]),
            "Hs": np.ascontiguousarray(H_share[bs]),
            "uT": np.ascontiguousarray(u[bs].T),
            "h_in": h[bs],
            "oh": np.ascontiguousarray(onehot[bs]),
            "ohT": np.ascontiguousarray(onehot[bs].T),
        })
    return in_maps


def _shared_weights(Wq, bq, Wki, bki, Wvi, bvi, Wks, bks, Wvs, bvs,
                    Wf, bf, Wgb, bgb):
    f = lambda x: np.ascontiguousarray(np.asarray(x, dtype=np.float32))
    Wq, Wki, Wvi, Wks, Wvs, Wf, Wgb = map(f, (Wq, Wki, Wvi, Wks, Wvs, Wf, Wgb))
    bq, bvi, bvs, bf, bgb = map(f, (bq, bvi, bvs, bf, bgb))
    return {
        "Wqu": Wq[0:DU],
        "Wqc": Wq[DU:DU + DH],
        "WkiT": np.ascontiguousarray(Wki.T),
        "WksT": np.ascontiguousarray(Wks.T),
        "Wvi": Wvi,
        "Wvs": Wvs,
        "Wf": Wf,
        "Wgbg": np.ascontiguousarray(np.vstack([Wgb[:, 0:DH], bgb[None, 0:DH]])),
        "Wgbb": np.ascontiguousarray(np.vstack([Wgb[:, DH:2 * DH],
                                                bgb[None, DH:2 * DH]])),
        "bq1": bq.reshape(QK, 1),
        "bvi1": bvi.reshape(QK, 1),
        "bvs1": bvs.reshape(QK, 1),
        "bf1": bf.reshape(CD, 1),
        "bfrep": np.tile(bf.reshape(1, CD), (P, 1)),
        "I128": np.eye(P, dtype=np.float32),
        "I128b": np.eye(P, dtype=np.float32).astype(ml_dtypes.bfloat16),
    }


def run(trace=False, **inputs):
    nc = _get_program()
    in_maps = _prep_in_maps(inputs["u"], inputs["H_intra"], inputs["H_share"],
                            inputs["h"], inputs["domain_ids"])
    shared = _shared_weights(
        inputs["Wq"], inputs["bq"], inputs["Wki"], inputs["bki"],
        inputs["Wvi"], inputs["bvi"], inputs["Wks"], inputs["bks"],
        inputs["Wvs"], inputs["bvs"], inputs["Wf"], inputs["bf"],
        inputs["Wgb"], inputs["bgb"])
    for m in in_maps:
        m.update(shared)
    res = run_bass_kernel_spmd(nc, in_maps, list(range(N_CORES)),
                               trace=trace)
    out = np.concatenate([res.results[c]["out"] for c in range(N_CORES)],
                         axis=0)
    l_con = np.float32(res.results[0]["lcon"][0, 0])
    return (out, l_con), res


def kernel(**inputs):
    (out, l_con), _ = run(trace=False, **inputs)
    return out, l_con


# revision 20
# speedup vs baseline: 1.5975x; 1.1629x over previous
"""Trainium2 Bass kernel for nn_CDIRecModel (CDI extractor + contrastive loss +
domain-masked AdaNorm), data-parallel over batch on 8 NeuronCores.

Key algebraic restructuring vs the reference:
  - Ki/Vi/Ks/Vs are never materialized.  Since
        softmax(Q @ (H W + b)^T) == softmax((H (W Q))^T)   (bias shift cancels)
    the intra/shared attention needs only kq = Q @ W^T per (d, b), then
        scores[b, l] = <H[b, l, :], kq[b, :]>          (fused DVE dot per l)
        hbar[b, :]   = sum_l softmax_l * H[b, l, :]    (fused DVE mul-add per l)
        h_att        = hbar @ Wv + bv                  (PE)
    This turns ~18 GFLOP of projections into ~0.7 GFLOP of fused vector work
    plus small matmuls, leaving the kernel HBM/DVE bound.
  - The per-(d,b) context mean over L is 50 accumulating PE transpose-matmuls
    (lhsT = H_l, rhs = I) into one PSUM tile -> c^T directly in the layout the
    Q projection wants.
  - Per-domain segment stats (sum h, sum h^2, count) and the contrastive-loss
    partial sum are computed as onehot matmuls into persistent PSUM, packed
    into one [6, 257] tile and AllReduced across the 8 cores for exact parity
    with the full-batch reference statistics.
"""

import sys

sys.path.insert(0, "/opt/trn_rl_repo")

import ml_dtypes
import numpy as np

import concourse.bass as bass
import concourse.bacc as bacc
import concourse.tile as tile
import concourse.mybir as mybir
from concourse.bass_utils import run_bass_kernel_spmd

# Problem shapes (hardcoded per contract)
D, B, L, LS, DH, DU, QK, CD = 5, 2048, 50, 50, 128, 128, 64, 64
EPS = 1e-5
N_CORES = 8
BC = B // N_CORES          # 256 batch rows per core
P = 128                    # partitions
NBLK = BC // P             # 2 blocks per core
SCALE = 0.125              # 1 / sqrt(QK)

F32 = mybir.dt.float32
BF16 = mybir.dt.bfloat16
AX = mybir.AxisListType
ALU = mybir.AluOpType
ACT = mybir.ActivationFunctionType

_CACHED = None


def _build_program():
    nc = bacc.Bacc("TRN2", target_bir_lowering=False, debug=False,
                   num_devices=N_CORES)

    def inp(name, shape):
        return nc.dram_tensor(name, shape, F32, kind="ExternalInput").ap()

    Hi = inp("Hi", [D, BC, L, DH])
    Hs = inp("Hs", [BC, LS, DH])
    uT = inp("uT", [DU, BC])
    h_in = inp("h_in", [BC, DH])
    oh = inp("oh", [BC, D])
    ohT = inp("ohT", [D, BC])
    Wqu = inp("Wqu", [DU, QK])
    Wqc = inp("Wqc", [DH, QK])
    WkiT = inp("WkiT", [QK, DH])
    WksT = inp("WksT", [QK, DH])
    Wvi = inp("Wvi", [DH, QK])
    Wvs = inp("Wvs", [DH, QK])
    Wf = inp("Wf", [2 * QK, CD])
    Wgbg = inp("Wgbg", [CD + 1, DH])   # [Wgb[:, :DH]; bgb[:DH]]
    Wgbb = inp("Wgbb", [CD + 1, DH])   # [Wgb[:, DH:]; bgb[DH:]]
    bq1 = inp("bq1", [QK, 1])
    bvi1 = inp("bvi1", [QK, 1])
    bvs1 = inp("bvs1", [QK, 1])
    bf1 = inp("bf1", [CD, 1])
    bfrep = inp("bfrep", [P, CD])
    I128 = inp("I128", [P, P])
    I128b = nc.dram_tensor("I128b", [P, P], BF16, kind="ExternalInput").ap()

    out = nc.dram_tensor("out", [BC, DH], F32, kind="ExternalOutput").ap()
    lcon = nc.dram_tensor("lcon", [1, 1], F32, kind="ExternalOutput").ap()

    with tile.TileContext(nc) as tc:
        with (
            tc.tile_pool(name="const", bufs=1) as const,
            tc.tile_pool(name="hbig", bufs=3) as hbig,
            tc.tile_pool(name="hsp", bufs=2) as hsp,
            tc.tile_pool(name="work", bufs=2) as work,
            tc.tile_pool(name="keep", bufs=2) as keep,
            tc.tile_pool(name="pwork", bufs=6, space="PSUM") as pwork,
            tc.tile_pool(name="ppers", bufs=1, space="PSUM") as ppers,
            tc.tile_pool(name="dram", bufs=1, space="DRAM") as dram,
        ):
            # ---- constants / weights to SBUF ----
            def cload(ap_in, shape):
                t = const.tile(shape, F32, tag=ap_in.tensor.name)
                nc.sync.dma_start(out=t[:], in_=ap_in)
                return t

            I_t = cload(I128, [P, P])
            Ib_t = const.tile([P, P], BF16, tag="Ib")
            nc.sync.dma_start(out=Ib_t[:], in_=I128b)
            Wqu_t = cload(Wqu, [DU, QK])
            Wqc_t = cload(Wqc, [DH, QK])
            WkiT_t = cload(WkiT, [QK, DH])
            WksT_t = cload(WksT, [QK, DH])
            Wvi_t = cload(Wvi, [DH, QK])
            Wvs_t = cload(Wvs, [DH, QK])
            Wf_t = cload(Wf, [2 * QK, CD])
            Wgbg_t = cload(Wgbg, [CD + 1, DH])
            Wgbb_t = cload(Wgbb, [CD + 1, DH])
            bq1_t = cload(bq1, [QK, 1])
            bvi1_t = cload(bvi1, [QK, 1])
            bvs1_t = cload(bvs1, [QK, 1])
            bf1_t = cload(bf1, [CD, 1])
            bfrep_t = cload(bfrep, [P, CD])
            ones_t = const.tile([P, 1], F32, tag="ones")
            nc.vector.memset(ones_t[:], 1.0)
            eps5_t = const.tile([D, 1], F32, tag="eps5")
            nc.vector.memset(eps5_t[:], EPS)

            # persistent PSUM accumulators (across both blocks)
            # cols: [0:DH] sum h, [DH:2DH] sum h^2, [2DH] count,
            # [2DH+1] (partition 0 only) contrastive-loss partial
            stats_blk = [ppers.tile([D, 2 * DH + 2], F32, tag=f"stats{i}",
                                    name=f"stats{i}")
                         for i in range(NBLK)]

            gammas, betas = [], []

            for blk in range(NBLK):
                b0 = blk * P
                hs_t = hsp.tile([P, LS, DH], BF16, tag="hs")
                nc.gpsimd.dma_start(out=hs_t[:], in_=Hs[b0:b0 + P, :, :])
                uT_t = work.tile([DU, P], F32, tag="uT")
                nc.sync.dma_start(out=uT_t[:], in_=uT[:, b0:b0 + P])
                h_t = keep.tile([P, DH], F32, tag="h")
                nc.sync.dma_start(out=h_t[:], in_=h_in[b0:b0 + P, :])
                oh_t = keep.tile([P, D], F32, tag="oh")
                nc.sync.dma_start(out=oh_t[:], in_=oh[b0:b0 + P, :])

                zT_all = keep.tile([QK, D, 2, P], F32, tag="zT_all")
                rdi_all = keep.tile([P, D], F32, tag="rdi_all")
                rds_all = keep.tile([P, D], F32, tag="rds_all")
                cat_all = keep.tile([P, D, 2 * QK], F32, tag="cat_all")
                nsq_i = keep.tile([P, D], F32, tag="nsq_i")
                nsq_s = keep.tile([P, D], F32, tag="nsq_s")
                posd = keep.tile([P, D], F32, tag="posd")

                def ph1(d):
                    """Load + mean + queries + scores for tile d."""
                    hb_t = hbig.tile([P, L, DH], BF16, tag="hb", name="hb")
                    nc.gpsimd.dma_start(out=hb_t[:],
                                        in_=Hi[d, b0:b0 + P, :, :])

                    # c^T = (1/L) * sum_l H_l^T  (PE transpose-accumulate)
                    cT_ps = pwork.tile([P, P], F32, tag="pp", name="cT_ps")
                    for l in range(L):
                        nc.tensor.matmul(cT_ps[:], hb_t[:, l, :], Ib_t[:],
                                         start=(l == 0), stop=(l == L - 1))
                    cT_sb = work.tile([P, P], F32, tag="cT", name="cT_sb")
                    nc.scalar.activation(cT_sb[:], cT_ps[:], ACT.Copy,
                                         scale=1.0 / L)

                    # Q^T = Wqu^T u^T + Wqc^T c^T + bq
                    QT_ps = pwork.tile([QK, P], F32, tag="pp", name="QT_ps")
                    nc.tensor.matmul(QT_ps[:], Wqu_t[:], uT_t[:],
                                     start=True, stop=False)
                    nc.tensor.matmul(QT_ps[:], Wqc_t[:], cT_sb[:],
                                     start=False, stop=True)
                    QT_sb = work.tile([QK, P], F32, tag="QT", name="QT_sb")
                    nc.scalar.activation(QT_sb[:], QT_ps[:], ACT.Identity,
                                         bias=bq1_t[:])

                    # kq = Q @ Wki^T ; kqs = Q @ Wks^T  (b-partition, bf16)
                    kq_ps = pwork.tile([P, DH], F32, tag="pp", name="kq_ps")
                    nc.tensor.matmul(kq_ps[:], QT_sb[:], WkiT_t[:],
                                     start=True, stop=True)
                    kq_sb = work.tile([P, DH], BF16, tag="kq", name="kq_sb")
                    nc.scalar.activation(kq_sb[:], kq_ps[:], ACT.Identity)
                    kqs_ps = pwork.tile([P, DH], F32, tag="pp", name="kqs_ps")
                    nc.tensor.matmul(kqs_ps[:], QT_sb[:], WksT_t[:],
                                     start=True, stop=True)
                    kqs_sb = work.tile([P, DH], BF16, tag="kqs",
                                       name="kqs_sb")
                    nc.scalar.activation(kqs_sb[:], kqs_ps[:], ACT.Identity)

                    st = {"d": d, "hb": hb_t}
                    for H_t, nl, kq_bf, sfx in ((hb_t, L, kq_sb, "i"),
                                                (hs_t, LS, kqs_sb, "s")):
                        tmp = work.tile([P, nl, DH], BF16, tag="tmp" + sfx,
                                        name="tmp")
                        kq_b = bass.AP(tensor=kq_bf[:].tensor,
                                       offset=kq_bf[:].offset,
                                       ap=[kq_bf[:].ap[0], [0, nl],
                                           kq_bf[:].ap[1]])
                        nc.vector.tensor_mul(tmp[:], H_t[:], kq_b)
                        s = work.tile([P, nl], F32, tag="s" + sfx, name="s")
                        nc.vector.reduce_sum(s[:], tmp[:], axis=AX.X)
                        m = work.tile([P, 1], F32, tag="m" + sfx, name="m")
                        nc.vector.reduce_max(m[:], s[:], axis=AX.X)
                        negm = work.tile([P, 1], F32, tag="negm" + sfx,
                                         name="negm")
                        nc.vector.tensor_scalar_mul(negm[:], m[:], -SCALE)
                        st[sfx] = (tmp, s, negm)
                    return st

                def ph2(st):
                    """exp + unnormalized attention sums + z/cat for tile."""
                    d = st["d"]
                    catT = work.tile([2 * QK, P], F32, tag="catT",
                                     name="catT")
                    for nl, H_t, Wv_t, c0, rall, sfx in (
                            (L, st["hb"], Wvi_t, 0, rdi_all, "i"),
                            (LS, hs_t, Wvs_t, QK, rds_all, "s")):
                        tmp, s, negm = st[sfx]
                        e_rep = work.tile([P, nl, DH], BF16, tag="erep",
                                          name="e_rep")
                        den = work.tile([P, 1], F32, tag="den" + sfx,
                                        name="den")
                        s_b = bass.AP(tensor=s[:].tensor, offset=s[:].offset,
                                      ap=[s[:].ap[0], s[:].ap[1], [0, DH]])
                        nc.scalar.activation(e_rep[:], s_b, ACT.Exp,
                                             bias=negm[:], scale=SCALE,
                                             accum_out=den[:])
                        nc.vector.tensor_mul(tmp[:], H_t[:], e_rep[:])
                        rden = work.tile([P, 1], F32, tag="rden" + sfx,
                                         name="rden")
                        nc.vector.reciprocal(rden[:], den[:])
                        nc.vector.tensor_scalar_mul(rall[:, d:d + 1],
                                                    rden[:], float(DH))
                        hbT_ps = pwork.tile([P, P], F32, tag="pp",
                                            name="hbT_ps")
                        for l in range(nl):
                            nc.tensor.matmul(hbT_ps[:], tmp[:, l, :], Ib_t[:],
                                             start=(l == 0),
                                             stop=(l == nl - 1))
                        hbT_sb = work.tile([P, P], F32, tag="hbT" + sfx,
                                           name="hbT_sb")
                        nc.scalar.activation(hbT_sb[:], hbT_ps[:],
                                             ACT.Identity)
                        hT_ps = pwork.tile([QK, P], F32, tag="pp",
                                           name="hT_ps")
                        nc.tensor.matmul(hT_ps[:], Wv_t[:], hbT_sb[:],
                                         start=True, stop=True)
                        nc.scalar.activation(catT[c0:c0 + QK, :], hT_ps[:],
                                             ACT.Identity)

                    # raw z^T halves (normalization deferred to zsel)
                    zTi_ps = pwork.tile([CD, P], F32, tag="pp", name="zTi_ps")
                    nc.tensor.matmul(zTi_ps[:], Wf_t[0:QK, :], catT[0:QK, :],
                                     start=True, stop=True)
                    nc.scalar.activation(zT_all[:, d, 0, :], zTi_ps[:],
                                         ACT.Identity)
                    zTs_ps = pwork.tile([CD, P], F32, tag="pp", name="zTs_ps")
                    nc.tensor.matmul(zTs_ps[:], Wf_t[QK:2 * QK, :],
                                     catT[QK:2 * QK, :],
                                     start=True, stop=True)
                    nc.scalar.activation(zT_all[:, d, 1, :], zTs_ps[:],
                                         ACT.Identity)

                    # cat back to b-partition layout for the dots
                    cat_ps = pwork.tile([P, P], F32, tag="pp", name="cat_ps")
                    nc.tensor.matmul(cat_ps[:], catT[:], I_t[:],
                                     start=True, stop=True)
                    nc.scalar.activation(cat_all[:, d, :], cat_ps[:],
                                         ACT.Identity)

                    junkd = work.tile([P, QK], F32, tag="junkd", name="junkd")
                    hi_ap = cat_all[:, d, 0:QK]
                    hs_ap = cat_all[:, d, QK:2 * QK]
                    nc.vector.scalar_tensor_tensor(
                        out=junkd[:], in0=hi_ap, scalar=1.0, in1=hi_ap,
                        op0=ALU.mult, op1=ALU.mult,
                        accum_out=nsq_i[:, d:d + 1])
                    nc.vector.scalar_tensor_tensor(
                        out=junkd[:], in0=hs_ap, scalar=1.0, in1=hs_ap,
                        op0=ALU.mult, op1=ALU.mult,
                        accum_out=nsq_s[:, d:d + 1])
                    nc.vector.scalar_tensor_tensor(
                        out=junkd[:], in0=hi_ap, scalar=1.0, in1=hs_ap,
                        op0=ALU.mult, op1=ALU.mult,
                        accum_out=posd[:, d:d + 1])

                # software pipeline: tile d's scores overlap tile d-1's
                # exp/attention-sum (DVE and ACT are each in-order)
                prev = None
                for d in range(D):
                    cur = ph1(d)
                    if prev is not None:
                        ph2(prev)
                    prev = cur
                ph2(prev)

                # ================= per-block epilogue =================
                # contrastive: rn = 1/max(sqrt(nsq), 1e-12)
                def recip_norm(nsq, sfx):
                    sq = work.tile([P, D], F32, tag="sq" + sfx)
                    nc.scalar.activation(sq[:], nsq[:], ACT.Sqrt)
                    nc.vector.tensor_scalar_max(sq[:], sq[:], 1e-12)
                    rn = keep.tile([P, D], F32, tag="rn" + sfx)
                    nc.vector.reciprocal(rn[:], sq[:])
                    return rn

                rni = recip_norm(nsq_i, "i")
                rns = recip_norm(nsq_s, "s")

                dots = keep.tile([P, D * D], F32, tag="dots")
                junkd2 = work.tile([P, QK], F32, tag="junkd2")
                for d in range(D):
                    for e2 in range(d, D):
                        nc.vector.scalar_tensor_tensor(
                            out=junkd2[:], in0=cat_all[:, d, 0:QK],
                            scalar=1.0, in1=cat_all[:, e2, 0:QK],
                            op0=ALU.mult, op1=ALU.mult,
                            accum_out=dots[:, d * D + e2:d * D + e2 + 1])
                for d in range(D):
                    for e2 in range(d):
                        nc.vector.tensor_copy(
                            dots[:, d * D + e2:d * D + e2 + 1],
                            dots[:, e2 * D + d:e2 * D + d + 1])
                # sim[b, d, e] = dots * rni[d] * rni[e]
                sim = work.tile([P, D * D], F32, tag="sim")
                for d in range(D):
                    row = slice(d * D, (d + 1) * D)
                    nc.vector.tensor_scalar_mul(sim[:, row], dots[:, row],
                                                rni[:, d:d + 1])
                    nc.vector.tensor_mul(sim[:, row], sim[:, row], rni[:])
                Eall = work.tile([P, D * D], F32, tag="Eall")
                nc.scalar.activation(Eall[:], sim[:], ACT.Exp)
                den_con = work.tile([P, D], F32, tag="den_con")
                nc.vector.reduce_sum(
                    den_con[:],
                    Eall[:].rearrange("p (d e) -> p d e", d=D), axis=AX.X)
                posn = work.tile([P, D], F32, tag="posn")
                nc.vector.tensor_mul(posn[:], posd[:], rni[:])
                nc.vector.tensor_mul(posn[:], posn[:], rns[:])
                EP = work.tile([P, D], F32, tag="EP")
                nc.scalar.activation(EP[:], posn[:], ACT.Exp)
                Rc = work.tile([P, D], F32, tag="Rc")
                nc.vector.tensor_scalar_add(Rc[:], den_con[:], 1e-8)
                nc.vector.reciprocal(Rc[:], Rc[:])
                Fr = work.tile([P, D], F32, tag="Fr")
                nc.vector.tensor_mul(Fr[:], EP[:], Rc[:])
                nc.vector.tensor_scalar_add(Fr[:], Fr[:], 1e-8)
                LG = work.tile([P, D], F32, tag="LG")
                nc.scalar.activation(LG[:], Fr[:], ACT.Ln)
                tsum = work.tile([P, 1], F32, tag="tsum")
                nc.vector.reduce_sum(tsum[:], LG[:], axis=AX.X)
                stats_ps = stats_blk[blk]
                nc.tensor.matmul(stats_ps[0:1, 2 * DH + 1:2 * DH + 2],
                                 tsum[:], ones_t[:], start=True, stop=True)

                # injector partial stats (persistent PSUM accumulate)
                h2 = work.tile([P, DH], F32, tag="h2")
                nc.vector.tensor_mul(h2[:], h_t[:], h_t[:])
                nc.tensor.matmul(stats_ps[:, 0:DH], oh_t[:], h_t[:],
                                 start=True, stop=True)
                nc.tensor.matmul(stats_ps[:, DH:2 * DH], oh_t[:], h2[:],
                                 start=True, stop=True)
                nc.tensor.matmul(stats_ps[:, 2 * DH:2 * DH + 1], oh_t[:],
                                 ones_t[:], start=True, stop=True)

                # z_sel = sum_d onehot[:, d] * (rden_i*z_i + rden_s*z_s)
                #         + bf_rep              (b-partition layout)
                zsel = work.tile([P, CD], F32, tag="zsel")
                nc.vector.memset(zsel[:], 0.0)
                for d in range(D):
                    for st, rall in ((0, rdi_all), (1, rds_all)):
                        zd_ps = pwork.tile([P, CD], F32, tag="pp")
                        nc.tensor.matmul(zd_ps[:], zT_all[:, d, st, :],
                                         I_t[0:QK, 0:QK],
                                         start=True, stop=True)
                        wsel = work.tile([P, 1], F32, tag="wsel")
                        nc.vector.tensor_mul(wsel[:], oh_t[:, d:d + 1],
                                             rall[:, d:d + 1])
                        nc.vector.scalar_tensor_tensor(
                            out=zsel[:], in0=zd_ps[:], scalar=wsel[:],
                            in1=zsel[:], op0=ALU.mult, op1=ALU.add)
                nc.vector.tensor_add(zsel[:], zsel[:], bfrep_t[:])
                zselT_aug = work.tile([CD + 1, P], F32, tag="zselT")
                nc.vector.memset(zselT_aug[CD:CD + 1, :], 1.0)
                zT2_ps = pwork.tile([CD, P], F32, tag="pp")
                nc.tensor.matmul(zT2_ps[:], zsel[:], I_t[:],
                                 start=True, stop=True)
                nc.vector.tensor_copy(zselT_aug[0:CD, :], zT2_ps[:])
                gm_ps = pwork.tile([P, DH], F32, tag="pp")
                nc.tensor.matmul(gm_ps[:], zselT_aug[:], Wgbg_t[:],
                                 start=True, stop=True)
                gm_sb = keep.tile([P, DH], F32, tag="gamma")
                nc.vector.tensor_copy(gm_sb[:], gm_ps[:])
                bt_ps = pwork.tile([P, DH], F32, tag="pp")
                nc.tensor.matmul(bt_ps[:], zselT_aug[:], Wgbb_t[:],
                                 start=True, stop=True)
                bt_sb = keep.tile([P, DH], F32, tag="beta")
                nc.vector.tensor_copy(bt_sb[:], bt_ps[:])
                gammas.append(gm_sb)
                betas.append(bt_sb)

            # ================= collective =================
            AR = keep.tile([D, 2 * DH + 2], F32, tag="AR")
            nc.vector.tensor_copy(AR[:], stats_blk[0][:])
            nc.vector.tensor_add(AR[:], AR[:], stats_blk[1][:])
            bi = dram.tile([D, 2 * DH + 2], F32)
            bo = dram.tile([D, 2 * DH + 2], F32)
            nc.gpsimd.dma_start(bi[:], AR[:])
            nc.gpsimd.collective_compute(
                "AllReduce", ALU.add,
                replica_groups=[list(range(N_CORES))],
                ins=[bi.opt()], outs=[bo.opt()])
            ARG = keep.tile([D, 2 * DH + 2], F32, tag="ARG")
            nc.gpsimd.dma_start(ARG[:], bo[:])

            # ---- global stats -> mean_d / rstd_d ----
            cntc = work.tile([D, 1], F32, tag="cntc")
            nc.vector.tensor_scalar_max(cntc[:], ARG[0:D, 2 * DH:2 * DH + 1],
                                        1.0)
            rc = work.tile([D, 1], F32, tag="rc")
            nc.vector.reciprocal(rc[:], cntc[:])
            mean_d = work.tile([D, DH], F32, tag="mean_d")
            nc.vector.tensor_scalar_mul(mean_d[:], ARG[0:D, 0:DH], rc[:])
            msq = work.tile([D, DH], F32, tag="msq")
            nc.vector.tensor_scalar_mul(msq[:], ARG[0:D, DH:2 * DH], rc[:])
            var_d = work.tile([D, DH], F32, tag="var_d")
            nc.vector.tensor_mul(var_d[:], mean_d[:], mean_d[:])
            nc.vector.tensor_sub(var_d[:], msq[:], var_d[:])
            sd = work.tile([D, DH], F32, tag="sd")
            nc.scalar.activation(sd[:], var_d[:], ACT.Sqrt, bias=eps5_t[:])
            rstd_d = work.tile([D, DH], F32, tag="rstd_d")
            nc.vector.reciprocal(rstd_d[:], sd[:])

            lc_sb = work.tile([1, 1], F32, tag="lc")
            nc.vector.tensor_scalar_mul(lc_sb[:], ARG[0:1, 2 * DH + 1:2 * DH + 2],
                                        -1.0 / (B * D))
            nc.sync.dma_start(out=lcon, in_=lc_sb[:])

            # ---- final assembly per block ----
            for blk in range(NBLK):
                b0 = blk * P
                ohT_t = work.tile([D, P], F32, tag="ohT_f")
                nc.sync.dma_start(out=ohT_t[:], in_=ohT[:, b0:b0 + P])
                h_t2 = work.tile([P, DH], F32, tag="h_f")
                nc.sync.dma_start(out=h_t2[:], in_=h_in[b0:b0 + P, :])
                mu_ps = pwork.tile([P, DH], F32, tag="pp")
                nc.tensor.matmul(mu_ps[:], ohT_t[:], mean_d[:],
                                 start=True, stop=True)
                hm = work.tile([P, DH], F32, tag="hm")
                nc.vector.tensor_sub(hm[:], h_t2[:], mu_ps[:])
                rs_ps = pwork.tile([P, DH], F32, tag="pp")
                nc.tensor.matmul(rs_ps[:], ohT_t[:], rstd_d[:],
                                 start=True, stop=True)
                hn = work.tile([P, DH], F32, tag="hn")
                nc.vector.tensor_mul(hn[:], hm[:], rs_ps[:])
                t1 = work.tile([P, DH], F32, tag="t1")
                nc.vector.tensor_mul(t1[:], hn[:], gammas[blk][:])
                nc.vector.tensor_add(t1[:], t1[:], betas[blk][:])
                nc.vector.tensor_add(t1[:], t1[:], h_t2[:])
                nc.sync.dma_start(out=out[b0:b0 + P, :], in_=t1[:])

    nc.compile()
    return nc


def _get_program():
    global _CACHED
    if _CACHED is None:
        _CACHED = _build_program()
    return _CACHED


def _prep_in_maps(u, H_intra, H_share, h, domain_ids):
    u = np.ascontiguousarray(np.asarray(u, dtype=np.float32))
    H_intra = np.asarray(H_intra, dtype=np.float32)
    H_share = np.asarray(H_share, dtype=np.float32)
    h = np.ascontiguousarray(np.asarray(h, dtype=np.float32))
    ids = np.asarray(domain_ids).astype(np.int64)
    onehot = (ids[:, None] == np.arange(D)[None, :]).astype(np.float32)

    in_maps = []
    for c in range(N_CORES):
        bs = slice(c * BC, (c + 1) * BC)
        in_maps.append({
            "Hi": np.ascontiguousarray(H_intra[:, bs]),
            "Hs": np.ascontiguousarray(H_share[bs]),
            "uT": np.ascontiguousarray(u[bs].T),
            "h_in": h[bs],
            "oh": np.ascontiguousarray(onehot[bs]),
            "ohT": np.ascontiguousarray(onehot[bs].T),
        })
    return in_maps


def _shared_weights(Wq, bq, Wki, bki, Wvi, bvi, Wks, bks, Wvs, bvs,
                    Wf, bf, Wgb, bgb):
    f = lambda x: np.ascontiguousarray(np.asarray(x, dtype=np.float32))
    Wq, Wki, Wvi, Wks, Wvs, Wf, Wgb = map(f, (Wq, Wki, Wvi, Wks, Wvs, Wf, Wgb))
    bq, bvi, bvs, bf, bgb = map(f, (bq, bvi, bvs, bf, bgb))
    return {
        "Wqu": Wq[0:DU],
        "Wqc": Wq[DU:DU + DH],
        "WkiT": np.ascontiguousarray(Wki.T),
        "WksT": np.ascontiguousarray(Wks.T),
        "Wvi": Wvi,
        "Wvs": Wvs,
        "Wf": Wf,
        "Wgbg": np.ascontiguousarray(np.vstack([Wgb[:, 0:DH], bgb[None, 0:DH]])),
        "Wgbb": np.ascontiguousarray(np.vstack([Wgb[:, DH:2 * DH],
                                                bgb[None, DH:2 * DH]])),
        "bq1": bq.reshape(QK, 1),
        "bvi1": bvi.reshape(QK, 1),
        "bvs1": bvs.reshape(QK, 1),
        "bf1": bf.reshape(CD, 1),
        "bfrep": np.tile(bf.reshape(1, CD), (P, 1)),
        "I128": np.eye(P, dtype=np.float32),
        "I128b": np.eye(P, dtype=np.float32).astype(ml_dtypes.bfloat16),
    }


def run(trace=False, **inputs):
    nc = _get_program()
    in_maps = _prep_in_maps(inputs["u"], inputs["H_intra"], inputs["H_share"],
                            inputs["h"], inputs["domain_ids"])
    shared = _shared_weights(
        inputs["Wq"], inputs["bq"], inputs["Wki"], inputs["bki"],
        inputs["Wvi"], inputs["bvi"], inputs["Wks"], inputs["bks"],
        inputs["Wvs"], inputs["bvs"], inputs["Wf"], inputs["bf"],
        inputs["Wgb"], inputs["bgb"])
    for m in in_maps:
        m.update(shared)
    res = run_bass_kernel_spmd(nc, in_maps, list(range(N_CORES)),
                               trace=trace)
    out = np.concatenate([res.results[c]["out"] for c in range(N_CORES)],
                         axis=0)
    l_con = np.float32(res.results[0]["lcon"][0, 0])
    return (out, l_con), res


def kernel(**inputs):
    (out, l_con), _ = run(trace=False, **inputs)
    return out, l_con
